# revision 14
# baseline (speedup 1.0000x reference)
"""Trainium2 Bass kernel for nn_CompRes (2-layer dilated-conv + BiMamba blocks).

Sharding: 8 cores = (batch b in 2) x (direction fwd/bwd) x (d-half of the mamba
inner channels). Every selective-scan recurrence is core-local (independent per
(b, dir, d, n)), running along time via the DVE tensor_tensor_scan instruction.
The backward direction uses host-reversed inputs + flipped conv taps so one
uniform SPMD program runs on all 8 cores. Two AllGathers per layer exchange
partial xproj / out-proj contractions. Pre/post-mamba stages are cheap matmuls
computed replicated per sample.

Numerics: residual path fp32; branch mostly bf16 (LayerScale 1e-4 damps branch
noise). Scan state is fp32 internally with bf16 operands.

Self-contained: hardcodes all shapes; reads no files.
"""
import numpy as np
import ml_dtypes

import concourse.bass as bass
import concourse.tile as tile
from concourse import mybir
from concourse.vector_clock import ScopedClock
from concourse.bass_utils import run_bass_kernel_spmd

F32 = mybir.dt.float32
BF16 = mybir.dt.bfloat16
AF = mybir.ActivationFunctionType
OP = mybir.AluOpType
BF = ml_dtypes.bfloat16

CH, HID, DI, DS, DR, DCONV = 384, 96, 192, 16, 6, 4
B = 2
T = 4096
CK = 512                 # matmul/psum chunk
MEGA = 1024              # scan mega-chunk
NCK = T // CK
NMC = T // MEGA
SUB = MEGA // CK
NG = 12                  # d-groups of 8 per core (96 local d's)
EPS = 1e-5
N_CORES = 8
DBC = DR + 2 * DS        # 38

_PATCHED = False


def _patch_tail_drain():
    """This walrus build rejects >2 sync waits on a CTRL instruction; split the
    TileContext tail-drain waits into one instruction each."""
    global _PATCHED
    if _PATCHED:
        return
    _PATCHED = True

    def _drain_and_barrier(self, tick_clock, wait_clock):
        nc = self.nc
        drain_inst = nc.sync.drain()
        wait_clock.add_sem_waits(
            drain_inst.ins, ScopedClock({None: tick_clock.global_clock})
        )
        si = drain_inst.ins.sync_info
        waits = list(si.on_wait)
        if len(waits) > 1:
            si.on_wait = []
            for w in waits:
                ni = nc.sync.nop(nofuse=True)
                ni.ins.sync_info = mybir.SyncInfo(on_wait=[w], on_update=[])
        nc.all_engine_barrier()
        assert self.sems is not None
        popped = nc._tile_sem_poison_stack.pop()
        assert popped is self._sem_poison
        nc.clear_and_free_semaphores(list(self.sems.allocated().values()))
        nc.all_engine_barrier()

    tile.TileContext._drain_and_barrier = _drain_and_barrier


# ---------------------------------------------------------------------------
# host-side input prep
# ---------------------------------------------------------------------------

def _prep_inputs(x, params):
    x = np.asarray(x, np.float32)

    def bf(a):
        return np.ascontiguousarray(np.asarray(a, np.float32).astype(BF))

    def f32(a):
        return np.ascontiguousarray(np.asarray(a, np.float32))

    q8g = np.zeros((96, NG, 128), np.float32)
    q16b = np.zeros((DBC, 128), np.float32)
    q16c = np.zeros((DBC, 128), np.float32)
    r96 = np.zeros((128, NG, 96), np.float32)
    for n in range(16):
        for dl in range(8):
            q16b[DR + n, n * 8 + dl] = -1.0
            q16c[DR + DS + n, n * 8 + dl] = 1.0
            for g in range(NG):
                q8g[8 * g + dl, g, n * 8 + dl] = 1.0
                r96[n * 8 + dl, g, 8 * g + dl] = 1.0
    ones96 = np.ones((96, 1), np.float32)
    ones128 = np.ones((128, 1), np.float32)
    onesrow = np.ones((1, 128), np.float32)

    cores = []
    for b in range(B):
        for dirflag in (0, 1):
            for dh in (0, 1):
                d0 = dh * 96
                dsl = slice(d0, d0 + 96)
                inp = {
                    'x_res': f32(x[b]),
                    'x_scan': bf(x[b][:, ::-1] if dirflag else x[b]),
                    'q8g': bf(q8g), 'q16b': bf(q16b), 'q16c': bf(q16c),
                    'r96': bf(r96),
                    'ones96': ones96, 'ones128': ones128, 'onesrow': onesrow,
                }
                for li, lname in enumerate(['layer0', 'layer1']):
                    lp = params[lname]
                    mp = lp['mb'] if dirflag else lp['mf']
                    sfx = f'_l{li}'
                    w = np.asarray(lp['c1w'], np.float32)      # [96, 384, 3]
                    wf = w[:, :, ::-1]
                    if li == 0:
                        wA, wB = (wf if dirflag else w), None
                    else:
                        wA = np.zeros_like(w) if dirflag else w
                        wB = wf if dirflag else np.zeros_like(w)

                    def packc1(wm):
                        # [128, 9, 96]: slot (k*3+kt) holds lhsT rows kt*128..
                        out = np.zeros((128, 9, 96), np.float32)
                        for k in range(3):
                            lhsT = wm[:, :, k].T              # [384, 96]
                            for kt in range(3):
                                out[:, k * 3 + kt, :] = lhsT[kt * 128:(kt + 1) * 128]
                        return out

                    inp['c1wA' + sfx] = bf(packc1(wA))
                    if li == 1:
                        inp['c1wB' + sfx] = bf(packc1(wB))
                    inp['c1b' + sfx] = f32(np.asarray(lp['c1b'])[:, None])
                    inp['gn1w' + sfx] = f32(np.asarray(lp['gn1w'])[:, None])
                    inp['gn1b' + sfx] = f32(np.asarray(lp['gn1b'])[:, None])

                    in_w = np.asarray(mp['in_w'], np.float32)  # [384, 96]
                    inp['inwx' + sfx] = bf(in_w[dsl].T)
                    inp['inwz' + sfx] = bf(in_w[192 + d0:192 + d0 + 96].T)
                    inp['convw' + sfx] = f32(np.asarray(mp['conv_w'])[dsl, 0])
                    inp['convb' + sfx] = f32(np.asarray(mp['conv_b'])[dsl][:, None])
                    inp['xpw' + sfx] = bf(np.asarray(mp['xproj_w'])[:, dsl].T)
                    inp['dtw' + sfx] = bf(np.asarray(mp['dt_w'])[dsl].T)
                    inp['dtb' + sfx] = f32(-np.asarray(mp['dt_b'])[dsl][:, None])
                    # dt_buf holds ndt = -softplus(raw); fold the sign here
                    A = np.exp(np.asarray(mp['A_log'], np.float32)[dsl])
                    acol = np.zeros((128, NG), np.float32)
                    for g in range(NG):
                        for n in range(16):
                            for dl in range(8):
                                acol[n * 8 + dl, g] = A[8 * g + dl, n]
                    inp['acol' + sfx] = acol
                    inp['dd' + sfx] = f32(np.asarray(mp['D'])[dsl][:, None])
                    inp['outw' + sfx] = bf(np.asarray(mp['out_w'])[:, dsl].T)

                    c2w = np.asarray(lp['c2w'], np.float32)[:, :, 0]
                    inp['c2wf' + sfx] = bf(c2w[:, 0:96].T)
                    inp['c2wb' + sfx] = bf(c2w[:, 96:192].T)
                    c2b = np.asarray(lp['c2b'], np.float32)
                    inp['c2ba' + sfx] = f32(c2b[0:384].reshape(3, 128).T)
                    inp['c2bg' + sfx] = f32(c2b[384:768].reshape(3, 128).T)
                    inp['gn2w' + sfx] = f32(np.asarray(lp['gn2w'], np.float32).reshape(3, 128).T)
                    inp['gn2b' + sfx] = f32(np.asarray(lp['gn2b'], np.float32).reshape(3, 128).T)
                    inp['lsc' + sfx] = f32(np.asarray(lp['scale'], np.float32).reshape(3, 128).T)
                cores.append(inp)
    return cores


# ---------------------------------------------------------------------------
# device program
# ---------------------------------------------------------------------------

def _declare_ios(nc):
    d = {}

    def di(name, shape, dt):
        d[name] = nc.dram_tensor(name, shape, dt, kind="ExternalInput").ap()

    di('x_res', [CH, T], F32)
    di('x_scan', [CH, T], BF16)
    di('q8g', [96, NG, 128], BF16)
    di('q16b', [DBC, 128], BF16)
    di('q16c', [DBC, 128], BF16)
    di('r96', [128, NG, 96], BF16)
    di('ones96', [96, 1], F32)
    di('ones128', [128, 1], F32)
    di('onesrow', [1, 128], F32)
    for li in (0, 1):
        sfx = f'_l{li}'
        di('c1wA' + sfx, [128, 9, 96], BF16)
        if li == 1:
            di('c1wB' + sfx, [128, 9, 96], BF16)
        di('c1b' + sfx, [96, 1], F32)
        di('gn1w' + sfx, [96, 1], F32)
        di('gn1b' + sfx, [96, 1], F32)
        di('inwx' + sfx, [96, 96], BF16)
        di('inwz' + sfx, [96, 96], BF16)
        di('convw' + sfx, [96, 4], F32)
        di('convb' + sfx, [96, 1], F32)
        di('xpw' + sfx, [96, DBC], BF16)
        di('dtw' + sfx, [6, 96], BF16)
        di('dtb' + sfx, [96, 1], F32)
        di('acol' + sfx, [128, NG], F32)
        di('dd' + sfx, [96, 1], F32)
        di('outw' + sfx, [96, 96], BF16)
        di('c2wf' + sfx, [96, 768], BF16)
        di('c2wb' + sfx, [96, 768], BF16)
        di('c2ba' + sfx, [128, 3], F32)
        di('c2bg' + sfx, [128, 3], F32)
        di('gn2w' + sfx, [128, 3], F32)
        di('gn2b' + sfx, [128, 3], F32)
        di('lsc' + sfx, [128, 3], F32)
    d['out'] = nc.dram_tensor('out', [CH, T], F32, kind="ExternalOutput").ap()
    return d


def _split_excess_waits(nc, max_waits=1):
    """This walrus build caps sync waits per instruction; hoist extras onto
    same-engine NoOps inserted just before the instruction."""
    uid = [0]
    for f in nc.m.functions:
        for bb in f.blocks:
            out = []
            for ins in bb.instructions:
                si = ins.sync_info
                if si is not None and len(si.on_wait) > max_waits:
                    waits = list(si.on_wait)
                    keep = waits[:max_waits]
                    extra = waits[max_waits:]
                    for i in range(0, len(extra), max_waits):
                        uid[0] += 1
                        nop = mybir.InstNoOp(
                            name=f"wsplit-{uid[0]}", engine=ins.engine,
                            ins=[], outs=[],
                            sync_info=mybir.SyncInfo(
                                on_wait=extra[i:i + max_waits], on_update=[]))
                        out.append(nop)
                    si.on_wait = keep
                out.append(ins)
            bb.instructions[:] = out


def _build_program():
    _patch_tail_drain()
    nc = bass.Bass(num_devices=N_CORES)
    io = _declare_ios(nc)
    with tile.TileContext(nc) as tc:
        _emit(nc, tc, io)
    _split_excess_waits(nc)
    return nc


def _emit(nc, tc, io):
    from contextlib import ExitStack
    ctx = ExitStack()
    persist = ctx.enter_context(tc.tile_pool(name="persist", bufs=1))
    wpool = ctx.enter_context(tc.tile_pool(name="weights", bufs=1))
    stream = ctx.enter_context(tc.tile_pool(name="stream", bufs=3))
    mega = ctx.enter_context(tc.tile_pool(name="mega", bufs=2))
    mega1 = ctx.enter_context(tc.tile_pool(name="mega1", bufs=1))
    small = ctx.enter_context(tc.tile_pool(name="small", bufs=2))
    tiny = ctx.enter_context(tc.tile_pool(name="tiny", bufs=1))
    ps_mm = ctx.enter_context(tc.tile_pool(name="ps_mm", bufs=3, space="PSUM"))
    ps_st = ctx.enter_context(tc.tile_pool(name="ps_st", bufs=1, space="PSUM"))
    ps_rep = ctx.enter_context(tc.tile_pool(name="ps_rep", bufs=2, space="PSUM"))
    ps_y = ctx.enter_context(tc.tile_pool(name="ps_y", bufs=2, space="PSUM"))
    dram = ctx.enter_context(tc.tile_pool(name="dram", bufs=1, space="DRAM"))

    # ---- load weights/constants to SBUF once ----
    sb = {}
    for name, ap in io.items():
        if name in ('x_res', 'x_scan', 'out'):
            continue
        t_ = wpool.tile(list(ap.shape), ap.dtype, tag=name)
        nc.sync.dma_start(out=t_[:], in_=ap[:])
        sb[name] = t_

    pair_groups = [[0, 1], [2, 3], [4, 5], [6, 7]]
    b_groups = [[0, 1, 2, 3], [4, 5, 6, 7]]
    gnstate = {}

    v0_buf = None
    x1_dram = x1r_dram = None

    for li in (0, 1):
        sfx = f'_l{li}'
        dil = 2 ** li

        # ---------------- Phase A: conv1 + GN1 stats ----------------
        u_buf = persist.tile([96, T], BF16, tag="u")
        stats = persist.tile([96, 2 * NCK], F32, tag="gn1stats")
        halo = dil
        for c in range(NCK):
            ups = ps_mm.tile([96, CK], F32, tag="mm")
            n_mm = 9 if li == 0 else 18
            mi = 0
            for src_i in range(1 if li == 0 else 2):
                if li == 0:
                    src_dram = io['x_scan']
                    wname = 'c1wA' + sfx
                else:
                    src_dram = (x1_dram if src_i == 0 else x1r_dram)
                    wname = ('c1wA' if src_i == 0 else 'c1wB') + sfx
                for kt in range(3):
                    xs = stream.tile([128, CK + 2 * halo], BF16, tag="xs")
                    lo, hi = c * CK - halo, c * CK + CK + halo
                    plo, phi = max(lo, 0), min(hi, T)
                    if lo < 0:
                        nc.vector.memset(xs[:, 0:(plo - lo)], 0.0)
                    if hi > T:
                        nc.vector.memset(xs[:, (phi - lo):(hi - lo)], 0.0)
                    nc.sync.dma_start(
                        out=xs[:, (plo - lo):(phi - lo)],
                        in_=src_dram[kt * 128:(kt + 1) * 128, plo:phi])
                    for k in range(3):
                        nc.tensor.matmul(
                            ups[:], sb[wname][:, k * 3 + kt, :],
                            xs[:, k * dil:k * dil + CK],
                            start=(mi == 0), stop=(mi == n_mm - 1))
                        mi += 1
            nc.scalar.activation(
                out=u_buf[:, c * CK:(c + 1) * CK], in_=ups[:],
                func=AF.Identity, bias=sb['c1b' + sfx][:, 0:1],
                accum_out=stats[:, c:c + 1])
            sq = small.tile([96, CK], F32, tag="sqA", bufs=1)
            nc.scalar.activation(
                out=sq[:], in_=u_buf[:, c * CK:(c + 1) * CK],
                func=AF.Square, accum_out=stats[:, NCK + c:NCK + c + 1])

        red = tiny.tile([96, 2], F32, tag="gn1red")
        nc.vector.tensor_reduce(out=red[:, 0:1], in_=stats[:, 0:NCK],
                                axis=mybir.AxisListType.X, op=OP.add)
        nc.vector.tensor_reduce(out=red[:, 1:2], in_=stats[:, NCK:2 * NCK],
                                axis=mybir.AxisListType.X, op=OP.add)
        tot_ps = ps_st.tile([1, 2], F32, tag="stat")
        nc.tensor.matmul(tot_ps[:], sb['ones96'][:], red[:], start=True, stop=True)
        _gn_finalize(nc, tiny, ps_st, sb, tot_ps, 96 * T, 96, "gn1", gnstate)
        s_gn, t_gn = _gn_scale_bias(nc, tiny, sb['gn1w' + sfx], sb['gn1b' + sfx],
                                    96, "gn1", gnstate)
        for c in range(NCK):
            nc.scalar.activation(
                out=u_buf[:, c * CK:(c + 1) * CK],
                in_=u_buf[:, c * CK:(c + 1) * CK],
                func=AF.Gelu, bias=t_gn[:, 0:1], scale=s_gn[:, 0:1])

        # ---------------- Phase B: in-proj, dconv, xproj ----------------
        zs_buf = persist.tile([96, T], BF16, tag="zs")
        xm_buf = persist.tile([96, T + DCONV - 1], BF16, tag="xm")
        xms_buf = persist.tile([96, T], BF16, tag="xms")
        dbc_part = persist.tile([DBC, T], BF16, tag="dbcpart")
        nc.vector.memset(xm_buf[:, 0:DCONV - 1], 0.0)
        for c in range(NCK):
            cc = slice(c * CK, (c + 1) * CK)
            xm_ps = ps_mm.tile([96, CK], F32, tag="mm")
            nc.tensor.matmul(xm_ps[:], sb['inwx' + sfx][:], u_buf[:, cc],
                             start=True, stop=True)
            nc.scalar.copy(
                out=xm_buf[:, DCONV - 1 + c * CK:DCONV - 1 + (c + 1) * CK],
                in_=xm_ps[:])
            z_ps = ps_mm.tile([96, CK], F32, tag="mm")
            nc.tensor.matmul(z_ps[:], sb['inwz' + sfx][:], u_buf[:, cc],
                             start=True, stop=True)
            nc.scalar.activation(out=zs_buf[:, cc], in_=z_ps[:], func=AF.Silu)
            acc = small.tile([96, CK], F32, tag="dconv")
            w4 = sb['convw' + sfx]
            nc.vector.tensor_scalar_mul(
                out=acc[:], in0=xm_buf[:, c * CK:c * CK + CK], scalar1=w4[:, 0:1])
            for j in range(1, DCONV):
                nc.vector.scalar_tensor_tensor(
                    out=acc[:], in0=xm_buf[:, c * CK + j:c * CK + j + CK],
                    scalar=w4[:, j:j + 1], in1=acc[:], op0=OP.mult, op1=OP.add)
            nc.scalar.activation(out=xms_buf[:, cc], in_=acc[:],
                                 func=AF.Silu, bias=sb['convb' + sfx][:, 0:1])
            xp_ps = ps_mm.tile([DBC, CK], F32, tag="mm")
            nc.tensor.matmul(xp_ps[:], sb['xpw' + sfx][:], xms_buf[:, cc],
                             start=True, stop=True)
            nc.scalar.copy(out=dbc_part[:, cc], in_=xp_ps[:])

        # ---------------- AllGather #1: dbc pair partials ----------------
        dbc_in = dram.tile([DBC, T], BF16, tag="dbc_in")
        dbc_out = dram.tile([2, DBC, T], BF16, tag="dbc_out")
        nc.gpsimd.dma_start(out=dbc_in[:], in_=dbc_part[:])
        nc.gpsimd.collective_compute(
            "AllGather", OP.bypass, replica_groups=pair_groups,
            ins=[dbc_in[:]], outs=[dbc_out[:]])
        ga = persist.tile([96, T], BF16, tag="gat_a")
        gb = persist.tile([96, T], BF16, tag="gat_b")
        nc.sync.dma_start(out=ga[0:DBC, :], in_=dbc_out[0, :, :])
        nc.sync.dma_start(out=gb[0:DBC, :], in_=dbc_out[1, :, :])
        dbc = persist.tile([DBC, T], BF16, tag="dbc")
        nc.vector.tensor_add(dbc[:], ga[0:DBC, :], gb[0:DBC, :])

        # ---------------- Phase C: dt, dtx ----------------
        dt_buf = persist.tile([96, T], BF16, tag="dt")
        dtx_buf = persist.tile([96, T], BF16, tag="dtx")
        # no Softplus act table on this HW: ndt = ln(sigmoid(-raw)) = -softplus
        # (sign folded into acol / q16b host constants). fp32 intermediate.
        nlo = (NCK + 1) // 2
        sig_lo = persist.tile([96, nlo * CK], F32, tag="u", name="sig_lo")
        sig_hi = (persist.tile([96, (NCK - nlo) * CK], F32, tag="xm",
                               name="sig_hi") if NCK > nlo else None)

        def _sig_slice(c):
            if c < nlo:
                return sig_lo[:, c * CK:(c + 1) * CK]
            return sig_hi[:, (c - nlo) * CK:(c - nlo + 1) * CK]

        for c in range(NCK):
            dt_ps = ps_mm.tile([96, CK], F32, tag="mm")
            nc.tensor.matmul(dt_ps[:], sb['dtw' + sfx][:], dbc[0:DR, cc := slice(c * CK, (c + 1) * CK)],
                             start=True, stop=True)
            nc.scalar.activation(out=_sig_slice(c), in_=dt_ps[:],
                                 func=AF.Sigmoid, scale=-1.0,
                                 bias=sb['dtb' + sfx][:, 0:1])
        for c in range(NCK):
            cc = slice(c * CK, (c + 1) * CK)
            nc.scalar.activation(out=dt_buf[:, cc], in_=_sig_slice(c), func=AF.Ln)
            nc.vector.tensor_mul(dtx_buf[:, cc], dt_buf[:, cc], xms_buf[:, cc])

        # ---------------- Phase D/E: scan tiles, gate, out-proj ----------
        yo_buf = persist.tile([96, T], BF16, tag="yo")
        carry = persist.tile([128, NG], F32, tag="carry")
        for mc in range(NMC):
            brep = mega1.tile([128, MEGA], BF16, tag="brep")
            crep = mega1.tile([128, MEGA], BF16, tag="crep")
            for s in range(SUB):
                c = mc * SUB + s
                cc = slice(c * CK, (c + 1) * CK)
                sc = slice(s * CK, (s + 1) * CK)
                bps = ps_rep.tile([128, CK], F32, tag="rep")
                nc.tensor.matmul(bps[:], sb['q16b'][:], dbc[:, cc],
                                 start=True, stop=True)
                nc.scalar.copy(out=brep[:, sc], in_=bps[:])
                cps = ps_rep.tile([128, CK], F32, tag="rep")
                nc.tensor.matmul(cps[:], sb['q16c'][:], dbc[:, cc],
                                 start=True, stop=True)
                nc.scalar.copy(out=crep[:, sc], in_=cps[:])
            yps_list = {}
            for s in range(SUB):
                yps_list[mc * SUB + s] = ps_y.tile([96, CK], F32, tag="y", name=f"yps_{mc}_{s}")
            for g in range(NG):
                dA = mega.tile([128, MEGA], BF16, tag="dA")
                dBx = mega.tile([128, MEGA], BF16, tag="dBx")
                for s in range(SUB):
                    c = mc * SUB + s
                    cc = slice(c * CK, (c + 1) * CK)
                    sc = slice(s * CK, (s + 1) * CK)
                    dtr_ps = ps_rep.tile([128, CK], F32, tag="rep")
                    nc.tensor.matmul(dtr_ps[:], sb['q8g'][:, g, :],
                                     dt_buf[:, cc],
                                     start=True, stop=True)
                    nc.scalar.activation(out=dA[:, sc], in_=dtr_ps[:],
                                         func=AF.Exp,
                                         scale=sb['acol' + sfx][:, g:g + 1])
                    dtxr_ps = ps_rep.tile([128, CK], F32, tag="rep")
                    nc.tensor.matmul(dtxr_ps[:], sb['q8g'][:, g, :],
                                     dtx_buf[:, cc],
                                     start=True, stop=True)
                    nc.vector.tensor_mul(dBx[:, sc], dtxr_ps[:], brep[:, sc])
                h = mega.tile([128, MEGA], BF16, tag="h")
                init = 0.0 if mc == 0 else carry[:, g:g + 1]
                nc.vector.tensor_tensor_scan(
                    out=h[:], data0=dA[:], data1=dBx[:], initial=init,
                    op0=OP.mult, op1=OP.add)
                nc.vector.tensor_copy(out=carry[:, g:g + 1],
                                      in_=h[:, MEGA - 1:MEGA])
                hc = mega.tile([128, MEGA], BF16, tag="hc")
                nc.vector.tensor_mul(hc[:], h[:], crep[:])
                for s in range(SUB):
                    c = mc * SUB + s
                    sc = slice(s * CK, (s + 1) * CK)
                    nc.tensor.matmul(yps_list[c][:],
                                     sb['r96'][:, g, :], hc[:, sc],
                                     start=(g == 0), stop=(g == NG - 1))
            for s in range(SUB):
                c = mc * SUB + s
                cc = slice(c * CK, (c + 1) * CK)
                yt = small.tile([96, CK], BF16, tag="ygate")
                nc.vector.scalar_tensor_tensor(
                    out=yt[:], in0=xms_buf[:, cc], scalar=sb['dd' + sfx][:, 0:1],
                    in1=yps_list[c][:], op0=OP.mult, op1=OP.add)
                nc.vector.tensor_mul(yt[:], yt[:], zs_buf[:, cc])
                yo_ps = ps_mm.tile([96, CK], F32, tag="mm")
                nc.tensor.matmul(yo_ps[:], sb['outw' + sfx][:], yt[:],
                                 start=True, stop=True)
                nc.scalar.copy(out=yo_buf[:, cc], in_=yo_ps[:])

        # ---------------- AllGather #2: yo batch partials ----------------
        yo_in = dram.tile([96, T], BF16, tag="yo_in")
        yo_out = dram.tile([4, 96, T], BF16, tag="yo_out")
        nc.gpsimd.dma_start(out=yo_in[:], in_=yo_buf[:])
        nc.gpsimd.collective_compute(
            "AllGather", OP.bypass, replica_groups=b_groups,
            ins=[yo_in[:]], outs=[yo_out[:]])
        yf = persist.tile([96, T], BF16, tag="u")      # u is dead; reuse slot
        yb = persist.tile([96, T], BF16, tag="zs")     # zs dead after gating
        ga = persist.tile([96, T], BF16, tag="gat_a")
        gb = persist.tile([96, T], BF16, tag="gat_b")
        nc.sync.dma_start(out=ga[:], in_=yo_out[0, :, :])
        nc.sync.dma_start(out=gb[:], in_=yo_out[1, :, :])
        nc.vector.tensor_add(yf[:], ga[:], gb[:])
        ga2 = persist.tile([96, T], BF16, tag="gat_a")
        gb2 = persist.tile([96, T], BF16, tag="gat_b")
        nc.sync.dma_start(out=ga2[:], in_=yo_out[2, :, :])
        nc.sync.dma_start(out=gb2[:], in_=yo_out[3, :, :])
        nc.vector.tensor_add(yb[:], ga2[:], gb2[:])

        # ---------------- Phase F: conv2 + GLU + GN2 ----------------
        glu = [persist.tile([128, T], BF16, tag=f"glu{m}", name=f"glu{m}") for m in range(3)]
        statsA = persist.tile([128, 3 * NCK], F32, tag="gn2sA")
        statsB = persist.tile([128, 3 * NCK], F32, tag="gn2sB")
        for c in range(NCK):
            cc = slice(c * CK, (c + 1) * CK)
            for m in range(3):
                msl = slice(m * 128, (m + 1) * 128)
                gsl = slice(384 + m * 128, 384 + (m + 1) * 128)
                g_ps = ps_mm.tile([128, CK], F32, tag="mm")
                nc.tensor.matmul(g_ps[:], sb['c2wf' + sfx][:, gsl], yf[:, cc],
                                 start=True, stop=False)
                nc.tensor.matmul(g_ps[:], sb['c2wb' + sfx][:, gsl], yb[:, cc],
                                 start=False, stop=True)
                sig = small.tile([128, CK], BF16, tag="sig")
                nc.scalar.activation(out=sig[:], in_=g_ps[:], func=AF.Sigmoid,
                                     bias=sb['c2bg' + sfx][:, m:m + 1])
                a_ps = ps_mm.tile([128, CK], F32, tag="mm")
                nc.tensor.matmul(a_ps[:], sb['c2wf' + sfx][:, msl], yf[:, cc],
                                 start=True, stop=False)
                nc.tensor.matmul(a_ps[:], sb['c2wb' + sfx][:, msl], yb[:, cc],
                                 start=False, stop=True)
                nc.vector.scalar_tensor_tensor(
                    out=glu[m][:, cc], in0=a_ps[:],
                    scalar=sb['c2ba' + sfx][:, m:m + 1], in1=sig[:],
                    op0=OP.add, op1=OP.mult,
                    accum_out=statsA[:, m * NCK + c:m * NCK + c + 1])
                sq = small.tile([128, CK], F32, tag="sqA", bufs=1)
                nc.scalar.activation(out=sq[:], in_=glu[m][:, cc],
                                     func=AF.Square,
                                     accum_out=statsB[:, m * NCK + c:m * NCK + c + 1])

        redB = tiny.tile([128, 2], F32, tag="gn2red")
        tot2_ps = ps_st.tile([1, 2], F32, tag="stat")
        for m in range(3):
            nc.vector.tensor_reduce(out=redB[:, 0:1], in_=statsA[:, m * NCK:(m + 1) * NCK],
                                    axis=mybir.AxisListType.X, op=OP.add)
            nc.vector.tensor_reduce(out=redB[:, 1:2], in_=statsB[:, m * NCK:(m + 1) * NCK],
                                    axis=mybir.AxisListType.X, op=OP.add)
            nc.tensor.matmul(tot2_ps[:], sb['ones128'][:], redB[:],
                             start=(m == 0), stop=(m == 2))
        _gn_finalize(nc, tiny, ps_st, sb, tot2_ps, CH * T, 128, "gn2", gnstate)
        svecs = []
        for m in range(3):
            sv = _gn_scale_bias(nc, tiny, sb['gn2w' + sfx][:, m:m + 1],
                                sb['gn2b' + sfx][:, m:m + 1], 128, f"gn2_{m}",
                                gnstate, base="gn2",
                                lsc=sb['lsc' + sfx][:, m:m + 1])
            svecs.append(sv)

        # ---------------- Phase G: v, residual, next-layer input ---------
        if li == 0:
            v0_buf = [persist.tile([128, T], BF16, tag=f"v0_{m}", name=f"v0_{m}") for m in range(3)]
            x1_dram = dram.tile([CH, T], BF16, tag="x1")
            x1r_dram = dram.tile([CH, T], BF16, tag="x1r")
            for m in range(3):
                s2, t2 = svecs[m]
                for c in range(NCK):
                    cc = slice(c * CK, (c + 1) * CK)
                    nc.scalar.activation(out=v0_buf[m][:, cc], in_=glu[m][:, cc],
                                         func=AF.Identity, bias=t2[:, 0:1],
                                         scale=s2[:, 0:1])
                    xr = stream.tile([128, CK], F32, tag="xres")
                    nc.sync.dma_start(out=xr[:],
                                      in_=io['x_res'][m * 128:(m + 1) * 128, cc])
                    x1c = small.tile([128, CK], BF16, tag="x1c", bufs=1)
                    nc.vector.tensor_add(x1c[:], xr[:], v0_buf[m][:, cc])
                    nc.sync.dma_start(out=x1_dram[m * 128:(m + 1) * 128, cc],
                                      in_=x1c[:])
                    x1rc = small.tile([128, CK], BF16, tag="x1rc", bufs=1)
                    nc.vector.tensor_copy(out=x1rc[:], in_=x1c[:, ::-1])
                    nc.sync.dma_start(
                        out=x1r_dram[m * 128:(m + 1) * 128,
                                     T - (c + 1) * CK:T - c * CK],
                        in_=x1rc[:])
        else:
            for m in range(3):
                s2, t2 = svecs[m]
                for c in range(NCK):
                    cc = slice(c * CK, (c + 1) * CK)
                    v1 = small.tile([128, CK], BF16, tag="v1", bufs=1)
                    nc.scalar.activation(out=v1[:], in_=glu[m][:, cc],
                                         func=AF.Identity, bias=t2[:, 0:1],
                                         scale=s2[:, 0:1])
                    vs = small.tile([128, CK], F32, tag="vsum", bufs=1)
                    nc.vector.tensor_add(vs[:], v1[:], v0_buf[m][:, cc])
                    xr = stream.tile([128, CK], F32, tag="xres")
                    nc.sync.dma_start(out=xr[:],
                                      in_=io['x_res'][m * 128:(m + 1) * 128, cc])
                    of = small.tile([128, CK], F32, tag="ofin", bufs=1)
                    nc.vector.tensor_add(of[:], vs[:], xr[:])
                    nc.sync.dma_start(out=io['out'][m * 128:(m + 1) * 128, cc],
                                      in_=of[:])
    ctx.close()


def _gn_finalize(nc, tiny, ps_st, sb, tot_ps, nelem, parts, tag, gnstate):
    """psum [2,1] (sum, sumsq) -> broadcast sbuf [parts,2] = (mean, rstd)."""
    st = tiny.tile([1, 2], F32, tag=tag + "_st")
    nc.scalar.mul(out=st[:], in_=tot_ps[:], mul=1.0 / nelem)
    msq = tiny.tile([1, 1], F32, tag=tag + "_msq")
    nc.scalar.square(out=msq[:], in_=st[:, 0:1])
    var = tiny.tile([1, 1], F32, tag=tag + "_var")
    nc.vector.tensor_sub(var[:], st[:, 1:2], msq[:])
    eps = tiny.tile([1, 1], F32, tag=tag + "_eps")
    nc.vector.memset(eps[:], EPS)
    nc.scalar.activation(out=var[:], in_=var[:], func=AF.Sqrt, bias=eps[:, 0:1])
    nc.vector.reciprocal(out=var[:], in_=var[:])
    mr = tiny.tile([1, 2], F32, tag=tag + "_mr")
    nc.vector.tensor_copy(out=mr[:, 0:1], in_=st[:, 0:1])
    nc.vector.tensor_copy(out=mr[:, 1:2], in_=var[:])
    bc_ps = ps_st.tile([parts, 2], F32, tag="stat")
    nc.tensor.matmul(bc_ps[:], sb['onesrow'][:, 0:parts], mr[:],
                     start=True, stop=True)
    bc = tiny.tile([parts, 2], F32, tag=tag + "_bc")
    nc.vector.tensor_copy(out=bc[:], in_=bc_ps[:])
    gnstate[tag] = bc


def _gn_scale_bias(nc, tiny, w_ap, b_ap, parts, tag, gnstate, base=None, lsc=None):
    """out = in*s + t  ==  (in - mean)*rstd*w + b, optionally *lsc folded."""
    bc = gnstate[base or tag]
    s = tiny.tile([parts, 1], F32, tag=tag + "_s")
    nc.vector.tensor_mul(s[:], w_ap[:, 0:1], bc[:, 1:2])
    tneg = tiny.tile([parts, 1], F32, tag=tag + "_tn")
    nc.vector.scalar_tensor_tensor(
        out=tneg[:], in0=bc[:, 0:1], scalar=s[:, 0:1], in1=b_ap[:, 0:1],
        op0=OP.mult, op1=OP.subtract)
    if lsc is not None:
        nc.vector.tensor_mul(s[:], s[:], lsc[:, 0:1])
        nc.vector.tensor_mul(tneg[:], tneg[:], lsc[:, 0:1])
    t = tiny.tile([parts, 1], F32, tag=tag + "_t")
    nc.vector.tensor_scalar_mul(out=t[:], in0=tneg[:], scalar1=-1.0)
    return s, t


# ---------------------------------------------------------------------------
# entry point
# ---------------------------------------------------------------------------

_CACHED = {}


def kernel(x, params):
    x = np.asarray(x, np.float32)
    cores = _prep_inputs(x, params)
    if 'nc' not in _CACHED:
        _CACHED['nc'] = _build_program()
    nc = _CACHED['nc']
    res = run_bass_kernel_spmd(nc, cores, core_ids=list(range(N_CORES)))
    out = np.stack([res.results[0]['out'], res.results[4]['out']])
    return out.astype(np.float32)


if __name__ == '__main__':
    import jax
    jax.config.update('jax_platforms', 'cpu')
    import sys
    sys.path.insert(0, '/root/problem')
    import reference
    inputs = reference.setup_inputs()
    expected = np.asarray(reference.reference(**inputs))
    got = kernel(np.asarray(inputs['x']),
                 jax.tree.map(np.asarray, inputs['params']))
    err = np.abs(got - expected)
    print('max abs err', err.max(), 'out scale', np.abs(expected).max())
    print('rel fro', np.linalg.norm(got - expected) / np.linalg.norm(expected))


# revision 31
# speedup vs baseline: 108.3682x; 108.3682x over previous
"""Trainium2 Bass kernel for nn_CompRes (2-layer dilated-conv + BiMamba blocks).

Sharding: 8 cores = (batch b in 2) x (direction fwd/bwd) x (d-half of the mamba
inner channels). Every selective-scan recurrence is core-local (independent per
(b, dir, d, n)), running along time via the DVE tensor_tensor_scan instruction.
The backward direction uses host-reversed inputs + flipped conv taps so one
uniform SPMD program runs on all 8 cores. Two AllGathers per layer exchange
partial xproj / out-proj contractions. Pre/post-mamba stages are cheap matmuls
computed replicated per sample.

Numerics: residual path fp32; branch mostly bf16 (LayerScale 1e-4 damps branch
noise). Scan state is fp32 internally with bf16 operands.

Self-contained: hardcodes all shapes; reads no files.
"""
import numpy as np
import ml_dtypes

import concourse.bass as bass
import concourse.tile as tile
from concourse import mybir
from concourse.vector_clock import ScopedClock
from concourse.bass_utils import run_bass_kernel_spmd

F32 = mybir.dt.float32
BF16 = mybir.dt.bfloat16
AF = mybir.ActivationFunctionType
OP = mybir.AluOpType
BF = ml_dtypes.bfloat16

CH, HID, DI, DS, DR, DCONV = 384, 96, 192, 16, 6, 4
B = 2
T = 4096
CK = 512                 # matmul/psum chunk
MEGA = 2048              # scan mega-chunk
NCK = T // CK
NMC = T // MEGA
SUB = MEGA // CK
NG = 12                  # d-groups of 8 per core (96 local d's)
EPS = 1e-5
N_CORES = 8
DBC = DR + 2 * DS        # 38

_PATCHED = False


def _patch_tail_drain():
    """This walrus build rejects >2 sync waits on a CTRL instruction; split the
    TileContext tail-drain waits into one instruction each."""
    global _PATCHED
    if _PATCHED:
        return
    _PATCHED = True

    def _drain_and_barrier(self, tick_clock, wait_clock):
        nc = self.nc
        drain_inst = nc.sync.drain()
        wait_clock.add_sem_waits(
            drain_inst.ins, ScopedClock({None: tick_clock.global_clock})
        )
        si = drain_inst.ins.sync_info
        waits = list(si.on_wait)
        if len(waits) > 1:
            si.on_wait = []
            for w in waits:
                ni = nc.sync.nop(nofuse=True)
                ni.ins.sync_info = mybir.SyncInfo(on_wait=[w], on_update=[])
        nc.all_engine_barrier()
        assert self.sems is not None
        popped = nc._tile_sem_poison_stack.pop()
        assert popped is self._sem_poison
        nc.clear_and_free_semaphores(list(self.sems.allocated().values()))
        nc.all_engine_barrier()

    tile.TileContext._drain_and_barrier = _drain_and_barrier


# ---------------------------------------------------------------------------
# host-side input prep
# ---------------------------------------------------------------------------

def _prep_inputs(x, params):
    x = np.asarray(x, np.float32)

    def bf(a):
        return np.ascontiguousarray(np.asarray(a, np.float32).astype(BF))

    def f32(a):
        return np.ascontiguousarray(np.asarray(a, np.float32))

    r96 = np.zeros((128, NG, 96), np.float32)
    for n in range(16):
        for dl in range(8):
            for g in range(NG):
                r96[n * 8 + dl, g, 8 * g + dl] = 1.0
    ones96 = np.ones((96, 1), np.float32)
    ones128 = np.ones((128, 1), np.float32)
    onesrow = np.ones((1, 128), np.float32)

    cores = []
    for b in range(B):
        for dirflag in (0, 1):
            for dh in (0, 1):
                d0 = dh * 96
                dsl = slice(d0, d0 + 96)
                inp = {
                    'x_res': f32(x[b]),
                    'x_scan': bf(x[b][:, ::-1] if dirflag else x[b]),
                    'r96': bf(r96),
                    'ones96': ones96, 'ones128': ones128, 'onesrow': onesrow,
                }
                for li, lname in enumerate(['layer0', 'layer1']):
                    lp = params[lname]
                    mp = lp['mb'] if dirflag else lp['mf']
                    sfx = f'_l{li}'
                    w = np.asarray(lp['c1w'], np.float32)      # [96, 384, 3]
                    wf = w[:, :, ::-1]
                    if li == 0:
                        wA, wB = (wf if dirflag else w), None
                    else:
                        wA = np.zeros_like(w) if dirflag else w
                        wB = wf if dirflag else np.zeros_like(w)

                    def packc1(wm):
                        # [128, 9, 96]: slot (k*3+kt) holds lhsT rows kt*128..
                        out = np.zeros((128, 9, 96), np.float32)
                        for k in range(3):
                            lhsT = wm[:, :, k].T              # [384, 96]
                            for kt in range(3):
                                out[:, k * 3 + kt, :] = lhsT[kt * 128:(kt + 1) * 128]
                        return out

                    inp['c1wA' + sfx] = bf(packc1(wA))
                    if li == 1:
                        inp['c1wB' + sfx] = bf(packc1(wB))
                    inp['c1b' + sfx] = f32(np.asarray(lp['c1b'])[:, None])
                    inp['gn1w' + sfx] = f32(np.asarray(lp['gn1w'])[:, None])
                    inp['gn1b' + sfx] = f32(np.asarray(lp['gn1b'])[:, None])

                    in_w = np.asarray(mp['in_w'], np.float32)  # [384, 96]
                    inp['inwx' + sfx] = bf(in_w[dsl].T)
                    inp['inwz' + sfx] = bf(in_w[192 + d0:192 + d0 + 96].T)
                    inp['convw' + sfx] = f32(np.asarray(mp['conv_w'])[dsl, 0])
                    inp['convb' + sfx] = f32(np.asarray(mp['conv_b'])[dsl][:, None])
                    xpw = np.asarray(mp['xproj_w'], np.float32)[:, dsl].T.copy()
                    xpw[:, DR:DR + DS] *= -1.0   # dtx carries the other -1
                    inp['xpw' + sfx] = bf(xpw)
                    cw4 = np.asarray(mp['conv_w'], np.float32)[dsl, 0]  # [96,4]
                    cdiag = np.zeros((96, DCONV, 96), np.float32)
                    for j in range(DCONV):
                        np.fill_diagonal(cdiag[:, j, :], cw4[:, j])
                    inp['cdiag' + sfx] = bf(cdiag)
                    inp['dtw' + sfx] = bf(np.asarray(mp['dt_w'])[dsl].T)
                    inp['dtb' + sfx] = f32(-np.asarray(mp['dt_b'])[dsl][:, None])
                    # dt_buf holds ndt = -softplus(raw); fold the sign here
                    A = np.exp(np.asarray(mp['A_log'], np.float32)[dsl])
                    acol = np.zeros((128, NG), np.float32)
                    for g in range(NG):
                        for n in range(16):
                            for dl in range(8):
                                acol[n * 8 + dl, g] = A[8 * g + dl, n]
                    inp['acol' + sfx] = acol
                    inp['dd' + sfx] = f32(np.asarray(mp['D'])[dsl][:, None])
                    inp['outw' + sfx] = bf(np.asarray(mp['out_w'])[:, dsl].T)

                    c2w = np.asarray(lp['c2w'], np.float32)[:, :, 0]
                    inp['c2wf' + sfx] = bf(c2w[:, 0:96].T)
                    inp['c2wb' + sfx] = bf(c2w[:, 96:192].T)
                    c2b = np.asarray(lp['c2b'], np.float32)
                    inp['c2ba' + sfx] = f32(c2b[0:384].reshape(3, 128).T)
                    inp['c2bg' + sfx] = f32(c2b[384:768].reshape(3, 128).T)
                    inp['gn2w' + sfx] = f32(np.asarray(lp['gn2w'], np.float32).reshape(3, 128).T)
                    inp['gn2b' + sfx] = f32(np.asarray(lp['gn2b'], np.float32).reshape(3, 128).T)
                    inp['lsc' + sfx] = f32(np.asarray(lp['scale'], np.float32).reshape(3, 128).T)
                cores.append(inp)
    return cores


# ---------------------------------------------------------------------------
# device program
# ---------------------------------------------------------------------------

def _declare_ios(nc):
    d = {}

    def di(name, shape, dt):
        d[name] = nc.dram_tensor(name, shape, dt, kind="ExternalInput").ap()

    di('x_res', [CH, T], F32)
    di('x_scan', [CH, T], BF16)
    di('r96', [128, NG, 96], BF16)
    di('ones96', [96, 1], F32)
    di('ones128', [128, 1], F32)
    di('onesrow', [1, 128], F32)
    for li in (0, 1):
        sfx = f'_l{li}'
        di('c1wA' + sfx, [128, 9, 96], BF16)
        if li == 1:
            di('c1wB' + sfx, [128, 9, 96], BF16)
        di('c1b' + sfx, [96, 1], F32)
        di('gn1w' + sfx, [96, 1], F32)
        di('gn1b' + sfx, [96, 1], F32)
        di('inwx' + sfx, [96, 96], BF16)
        di('inwz' + sfx, [96, 96], BF16)
        di('convw' + sfx, [96, 4], F32)
        di('cdiag' + sfx, [96, DCONV, 96], BF16)
        di('convb' + sfx, [96, 1], F32)
        di('xpw' + sfx, [96, DBC], BF16)
        di('dtw' + sfx, [6, 96], BF16)
        di('dtb' + sfx, [96, 1], F32)
        di('acol' + sfx, [128, NG], F32)
        di('dd' + sfx, [96, 1], F32)
        di('outw' + sfx, [96, 96], BF16)
        di('c2wf' + sfx, [96, 768], BF16)
        di('c2wb' + sfx, [96, 768], BF16)
        di('c2ba' + sfx, [128, 3], F32)
        di('c2bg' + sfx, [128, 3], F32)
        di('gn2w' + sfx, [128, 3], F32)
        di('gn2b' + sfx, [128, 3], F32)
        di('lsc' + sfx, [128, 3], F32)
    d['out'] = nc.dram_tensor('out', [CH, T], F32, kind="ExternalOutput").ap()
    return d


def _split_excess_waits(nc, max_waits=1):
    """This walrus build caps sync waits per instruction; hoist extras onto
    same-engine NoOps inserted just before the instruction."""
    uid = [0]
    for f in nc.m.functions:
        for bb in f.blocks:
            out = []
            for ins in bb.instructions:
                si = ins.sync_info
                if si is not None and len(si.on_wait) > max_waits:
                    waits = list(si.on_wait)
                    keep = waits[:max_waits]
                    extra = waits[max_waits:]
                    for i in range(0, len(extra), max_waits):
                        uid[0] += 1
                        nop = mybir.InstNoOp(
                            name=f"wsplit-{uid[0]}", engine=ins.engine,
                            ins=[], outs=[],
                            sync_info=mybir.SyncInfo(
                                on_wait=extra[i:i + max_waits], on_update=[]))
                        out.append(nop)
                    si.on_wait = keep
                out.append(ins)
            bb.instructions[:] = out


def _build_program(single=False):
    _patch_tail_drain()
    nc = bass.Bass(num_devices=1 if single else N_CORES)
    io = _declare_ios(nc)
    with tile.TileContext(nc) as tc:
        _emit(nc, tc, io, single=single)
    _split_excess_waits(nc)
    return nc


def _emit(nc, tc, io, single=False):
    from contextlib import ExitStack
    ctx = ExitStack()
    persist = ctx.enter_context(tc.tile_pool(name="persist", bufs=1))
    wpool = ctx.enter_context(tc.tile_pool(name="weights", bufs=1))
    stream = ctx.enter_context(tc.tile_pool(name="stream", bufs=3))
    mega = ctx.enter_context(tc.tile_pool(name="mega", bufs=2))
    mega1 = ctx.enter_context(tc.tile_pool(name="mega1", bufs=1))
    small = ctx.enter_context(tc.tile_pool(name="small", bufs=2))
    tiny = ctx.enter_context(tc.tile_pool(name="tiny", bufs=1))
    ps_mm = ctx.enter_context(tc.tile_pool(name="ps_mm", bufs=3, space="PSUM"))
    ps_st = ctx.enter_context(tc.tile_pool(name="ps_st", bufs=1, space="PSUM"))
    ps_y = ctx.enter_context(tc.tile_pool(name="ps_y", bufs=4, space="PSUM"))
    dram = ctx.enter_context(tc.tile_pool(name="dram", bufs=1, space="DRAM"))

    # ---- load weights/constants to SBUF once ----
    sb = {}
    for name, ap in io.items():
        if name in ('x_res', 'x_scan', 'out'):
            continue
        t_ = wpool.tile(list(ap.shape), ap.dtype, tag=name)
        nc.sync.dma_start(out=t_[:], in_=ap[:])
        sb[name] = t_

    pair_groups = [[0, 1], [2, 3], [4, 5], [6, 7]]
    b_groups = [[0, 1, 2, 3], [4, 5, 6, 7]]
    gnstate = {}

    v0_buf = None
    x1_dram = None

    for li in (0, 1):
        sfx = f'_l{li}'
        dil = 2 ** li

        # ---------------- Phase A: conv1 + GN1 stats ----------------
        u_buf = persist.tile([96, T], BF16, tag="u")
        stats = persist.tile([96, 2 * NCK], F32, tag="gn1stats")
        halo = dil
        for c in range(NCK):
            ups = ps_mm.tile([96, CK], F32, tag="mm")
            n_mm = 9 if li == 0 else 18
            mi = 0
            if li == 0:
                for kt in range(3):
                    xs = stream.tile([128, CK + 2 * halo], BF16, tag="xs", bufs=2)
                    lo, hi = c * CK - halo, c * CK + CK + halo
                    plo, phi = max(lo, 0), min(hi, T)
                    if lo < 0:
                        nc.vector.memset(xs[:, 0:(plo - lo)], 0.0)
                    if hi > T:
                        nc.vector.memset(xs[:, (phi - lo):(hi - lo)], 0.0)
                    nc.sync.dma_start(
                        out=xs[:, (plo - lo):(phi - lo)],
                        in_=io['x_scan'][kt * 128:(kt + 1) * 128, plo:phi])
                    for k in range(3):
                        nc.tensor.matmul(
                            ups[:], sb['c1wA' + sfx][:, k * 3 + kt, :],
                            xs[:, k * dil:k * dil + CK],
                            start=(mi == 0), stop=(mi == n_mm - 1))
                        mi += 1
            else:
                for kt in range(3):
                    lo, hi = c * CK - halo, c * CK + CK + halo
                    plo, phi = max(lo, 0), min(hi, T)
                    xs = stream.tile([128, CK + 2 * halo], BF16, tag="xs", bufs=2)
                    if lo < 0:
                        nc.vector.memset(xs[:, 0:(plo - lo)], 0.0)
                    if hi > T:
                        nc.vector.memset(xs[:, (phi - lo):(hi - lo)], 0.0)
                    nc.sync.dma_start(
                        out=xs[:, (plo - lo):(phi - lo)],
                        in_=x1_dram[kt * 128:(kt + 1) * 128, plo:phi])
                    # reversed frame handled via reversed SBUF AP on the
                    # matmul rhs; tile holds x1 cols [rlo, rhi) contiguously.
                    rlo, rhi = T - hi, T - lo
                    rplo, rphi = max(rlo, 0), min(rhi, T)
                    xr_ = stream.tile([128, CK + 2 * halo], BF16, tag="xsr", bufs=2)
                    if rlo < 0:
                        nc.vector.memset(xr_[:, 0:(rplo - rlo)], 0.0)
                    if rhi > T:
                        nc.vector.memset(xr_[:, (rphi - rlo):(rhi - rlo)], 0.0)
                    nc.sync.dma_start(
                        out=xr_[:, (rplo - rlo):(rphi - rlo)],
                        in_=x1_dram[kt * 128:(kt + 1) * 128, rplo:rphi])
                    for k in range(3):
                        nc.tensor.matmul(
                            ups[:], sb['c1wA' + sfx][:, k * 3 + kt, :],
                            xs[:, k * dil:k * dil + CK],
                            start=(mi == 0), stop=False)
                        mi += 1
                        roff = halo - (k - 1) * dil
                        nc.tensor.matmul(
                            ups[:], sb['c1wB' + sfx][:, k * 3 + kt, :],
                            xr_[:, roff:roff + CK][:, ::-1],
                            start=False, stop=(mi == n_mm - 1))
                        mi += 1
            nc.scalar.activation(
                out=u_buf[:, c * CK:(c + 1) * CK], in_=ups[:],
                func=AF.Identity, bias=sb['c1b' + sfx][:, 0:1],
                accum_out=stats[:, c:c + 1])
            sq = small.tile([96, CK], F32, tag="sqA", bufs=1)
            nc.scalar.activation(
                out=sq[:], in_=u_buf[:, c * CK:(c + 1) * CK],
                func=AF.Square, accum_out=stats[:, NCK + c:NCK + c + 1])

        red = tiny.tile([96, 2], F32, tag="gn1red")
        nc.vector.tensor_reduce(out=red[:, 0:1], in_=stats[:, 0:NCK],
                                axis=mybir.AxisListType.X, op=OP.add)
        nc.vector.tensor_reduce(out=red[:, 1:2], in_=stats[:, NCK:2 * NCK],
                                axis=mybir.AxisListType.X, op=OP.add)
        tot_ps = ps_st.tile([1, 2], F32, tag="stat")
        nc.tensor.matmul(tot_ps[:], sb['ones96'][:], red[:], start=True, stop=True)
        _gn_finalize(nc, tiny, ps_st, sb, tot_ps, 96 * T, 96, "gn1", gnstate)
        s_gn, t_gn = _gn_scale_bias(nc, tiny, sb['gn1w' + sfx], sb['gn1b' + sfx],
                                    96, "gn1", gnstate)
        for c in range(NCK):
            nc.scalar.activation(
                out=u_buf[:, c * CK:(c + 1) * CK],
                in_=u_buf[:, c * CK:(c + 1) * CK],
                func=AF.Gelu, bias=t_gn[:, 0:1], scale=s_gn[:, 0:1])

        # ---------------- Phase B: in-proj, dconv, xproj ----------------
        zs_buf = persist.tile([96, T], BF16, tag="zs")
        xm_buf = persist.tile([96, T + DCONV - 1], BF16, tag="xm")
        xms_buf = persist.tile([96, T], BF16, tag="xms")
        dbc_part = persist.tile([DBC, T], BF16, tag="dt", name="dbc_part")
        nc.vector.memset(xm_buf[:, 0:DCONV - 1], 0.0)
        for c in range(NCK):
            cc = slice(c * CK, (c + 1) * CK)
            xm_ps = ps_mm.tile([96, CK], F32, tag="mm")
            nc.tensor.matmul(xm_ps[:], sb['inwx' + sfx][:], u_buf[:, cc],
                             start=True, stop=True)
            nc.vector.tensor_copy(
                out=xm_buf[:, DCONV - 1 + c * CK:DCONV - 1 + (c + 1) * CK],
                in_=xm_ps[:])
            z_ps = ps_mm.tile([96, CK], F32, tag="mm")
            nc.tensor.matmul(z_ps[:], sb['inwz' + sfx][:], u_buf[:, cc],
                             start=True, stop=True)
            nc.scalar.activation(out=zs_buf[:, cc], in_=z_ps[:], func=AF.Silu)
            # depthwise conv as 4 diag-matmuls accumulating in psum
            dc_ps = ps_mm.tile([96, CK], F32, tag="mm")
            for j in range(DCONV):
                nc.tensor.matmul(dc_ps[:], sb['cdiag' + sfx][:, j, :],
                                 xm_buf[:, c * CK + j:c * CK + j + CK],
                                 start=(j == 0), stop=(j == DCONV - 1))
            nc.scalar.activation(out=xms_buf[:, cc], in_=dc_ps[:],
                                 func=AF.Silu, bias=sb['convb' + sfx][:, 0:1])
            xp_ps = ps_mm.tile([DBC, CK], F32, tag="mm")
            nc.tensor.matmul(xp_ps[:], sb['xpw' + sfx][:], xms_buf[:, cc],
                             start=True, stop=True)
            nc.vector.tensor_copy(out=dbc_part[:, cc], in_=xp_ps[:])

        # ---------------- AllGather #1: dbc pair partials ----------------
        dbc_in = dram.tile([DBC, T], BF16, tag="dbc_in")
        dbc_out = dram.tile([2, DBC, T], BF16, tag="dbc_out")
        nc.gpsimd.dma_start(out=dbc_in[:], in_=dbc_part[:])
        if single:
            for _sl in range(2):
                nc.gpsimd.dma_start(out=dbc_out[_sl, :, :], in_=dbc_in[:])
        else:
            nc.gpsimd.collective_compute(
                "AllGather", OP.bypass, replica_groups=pair_groups,
                ins=[dbc_in[:]], outs=[dbc_out[:]])
        ga = persist.tile([96, T], BF16, tag="u", name="ga1")
        gb = persist.tile([96, T], BF16, tag="xm", name="gb1")
        nc.sync.dma_start(out=ga[0:DBC, :], in_=dbc_out[0, :, :])
        nc.sync.dma_start(out=gb[0:DBC, :], in_=dbc_out[1, :, :])
        dbc = persist.tile([DBC, T], BF16, tag="dbc")
        nc.vector.tensor_add(dbc[:], ga[0:DBC, :], gb[0:DBC, :])
        dbc_dram = dram.tile([DBC, T], BF16, tag="dbc_dram")
        nc.sync.dma_start(out=dbc_dram[:], in_=dbc[:])

        # ---------------- Phase C: dt, dtx ----------------
        dt_buf = persist.tile([96, T], BF16, tag="dt", name="dt_buf")
        dtx_buf = persist.tile([96, T], BF16, tag="dtx")
        # no Softplus act table on this HW: ndt = ln(sigmoid(-raw)) = -softplus
        # (sign folded into acol / q16b host constants). fp32 intermediate.
        nlo = (NCK + 1) // 2
        sig_lo = persist.tile([96, nlo * CK], F32, tag="u", name="sig_lo")
        sig_hi = (persist.tile([96, (NCK - nlo) * CK], F32, tag="xm",
                               name="sig_hi") if NCK > nlo else None)

        def _sig_slice(c):
            if c < nlo:
                return sig_lo[:, c * CK:(c + 1) * CK]
            return sig_hi[:, (c - nlo) * CK:(c - nlo + 1) * CK]

        for c in range(NCK):
            dt_ps = ps_mm.tile([96, CK], F32, tag="mm")
            nc.tensor.matmul(dt_ps[:], sb['dtw' + sfx][:], dbc[0:DR, cc := slice(c * CK, (c + 1) * CK)],
                             start=True, stop=True)
            nc.scalar.activation(out=_sig_slice(c), in_=dt_ps[:],
                                 func=AF.Sigmoid, scale=-1.0,
                                 bias=sb['dtb' + sfx][:, 0:1])
        dt_dram = dram.tile([96, T], BF16, tag="dt_dram")
        dtx_dram = dram.tile([96, T], BF16, tag="dtx_dram")
        for c in range(NCK):
            cc = slice(c * CK, (c + 1) * CK)
            nc.scalar.activation(out=dt_buf[:, cc], in_=_sig_slice(c), func=AF.Ln)
            nc.vector.tensor_mul(dtx_buf[:, cc], dt_buf[:, cc], xms_buf[:, cc])
            nc.gpsimd.dma_start(out=dt_dram[:, cc], in_=dt_buf[:, cc])
            nc.gpsimd.dma_start(out=dtx_dram[:, cc], in_=dtx_buf[:, cc])

        # ---------------- Phase D/E: scan tiles, gate, out-proj ----------
        yo_buf = persist.tile([96, T], BF16, tag="dtx", name="yo_buf")
        carry = persist.tile([128, NG], F32, tag="carry")
        dbc_b = dbc_dram[:]
        dt_b = dt_dram[:]
        dtx_b = dtx_dram[:]
        for mc in range(NMC):
            off = mc * MEGA
            brep = mega1.tile([128, MEGA], BF16, tag="brep")
            crep = mega1.tile([128, MEGA], BF16, tag="crep")
            nc.sync.dma_start(out=brep[:], in_=bass.AP(
                tensor=dbc_b.tensor, offset=dbc_b.offset + DR * T + off,
                ap=[[T, 16], [0, 8], [1, MEGA]]))
            nc.sync.dma_start(out=crep[:], in_=bass.AP(
                tensor=dbc_b.tensor, offset=dbc_b.offset + (DR + DS) * T + off,
                ap=[[T, 16], [0, 8], [1, MEGA]]))
            yps_list = {}
            for s in range(SUB):
                yps_list[mc * SUB + s] = ps_y.tile([96, CK], F32, tag="y", name=f"yps_{mc}_{s}")
            for g in range(NG):
                dA = mega.tile([128, MEGA], BF16, tag="dA", bufs=3)
                dBx = mega.tile([128, MEGA], BF16, tag="dBx", bufs=3)
                nc.scalar.dma_start(out=dA[:], in_=bass.AP(
                    tensor=dt_b.tensor, offset=dt_b.offset + (8 * g) * T + off,
                    ap=[[0, 16], [T, 8], [1, MEGA]]))
                nc.scalar.activation(out=dA[:], in_=dA[:], func=AF.Exp,
                                     scale=sb['acol' + sfx][:, g:g + 1])
                nc.scalar.dma_start(out=dBx[:], in_=bass.AP(
                    tensor=dtx_b.tensor, offset=dtx_b.offset + (8 * g) * T + off,
                    ap=[[0, 16], [T, 8], [1, MEGA]]))
                nc.vector.tensor_mul(dBx[:], dBx[:], brep[:])
                h = mega.tile([128, MEGA], BF16, tag="h")
                init = 0.0 if mc == 0 else carry[:, g:g + 1]
                nc.vector.tensor_tensor_scan(
                    out=h[:], data0=dA[:], data1=dBx[:], initial=init,
                    op0=OP.mult, op1=OP.add)
                nc.vector.tensor_copy(out=carry[:, g:g + 1],
                                      in_=h[:, MEGA - 1:MEGA])
                hc = mega.tile([128, MEGA], BF16, tag="hc", name=f"hc_{mc}_{g}")
                if g % 2 == 1:
                    nc.gpsimd.tensor_mul(hc[:], h[:], crep[:])
                else:
                    nc.vector.tensor_mul(hc[:], h[:], crep[:])
                for s in range(SUB):
                    c = mc * SUB + s
                    sc = slice(s * CK, (s + 1) * CK)
                    nc.tensor.matmul(yps_list[c][:],
                                     sb['r96'][:, g, :], hc[:, sc],
                                     start=(g == 0), stop=(g == NG - 1))
            for s in range(SUB):
                c = mc * SUB + s
                cc = slice(c * CK, (c + 1) * CK)
                yt = small.tile([96, CK], BF16, tag="ygate")
                nc.vector.scalar_tensor_tensor(
                    out=yt[:], in0=xms_buf[:, cc], scalar=sb['dd' + sfx][:, 0:1],
                    in1=yps_list[c][:], op0=OP.mult, op1=OP.add)
                nc.vector.tensor_mul(yt[:], yt[:], zs_buf[:, cc])
                yo_ps = ps_mm.tile([96, CK], F32, tag="mm")
                nc.tensor.matmul(yo_ps[:], sb['outw' + sfx][:], yt[:],
                                 start=True, stop=True)
                nc.scalar.copy(out=yo_buf[:, cc], in_=yo_ps[:])

        # ---------------- AllGather #2: yo batch partials ----------------
        yo_in = dram.tile([96, T], BF16, tag="yo_in")
        yo_out = dram.tile([4, 96, T], BF16, tag="yo_out")
        nc.gpsimd.dma_start(out=yo_in[:], in_=yo_buf[:])
        if single:
            for _sl in range(4):
                nc.gpsimd.dma_start(out=yo_out[_sl, :, :], in_=yo_in[:])
        else:
            nc.gpsimd.collective_compute(
                "AllGather", OP.bypass, replica_groups=b_groups,
                ins=[yo_in[:]], outs=[yo_out[:]])
        yf = persist.tile([96, T], BF16, tag="u", name="yf")
        yb = persist.tile([96, T], BF16, tag="zs", name="yb")
        ga3 = persist.tile([96, T], BF16, tag="xm", name="ga3")
        gb3 = persist.tile([96, T], BF16, tag="dtx", name="gb3")
        nc.sync.dma_start(out=ga3[:], in_=yo_out[0, :, :])
        nc.sync.dma_start(out=gb3[:], in_=yo_out[1, :, :])
        nc.vector.tensor_add(yf[:], ga3[:], gb3[:])
        ga4 = persist.tile([96, T], BF16, tag="xm", name="ga4")
        gb4 = persist.tile([96, T], BF16, tag="dtx", name="gb4")
        nc.sync.dma_start(out=ga4[:], in_=yo_out[2, :, :])
        nc.sync.dma_start(out=gb4[:], in_=yo_out[3, :, :])
        nc.vector.tensor_add(yb[:], ga4[:], gb4[:])

        # ---------------- Phase F: conv2 + GLU + GN2 ----------------
        glu = [persist.tile([128, T], BF16, tag=f"glu{m}", name=f"glu{m}") for m in range(3)]
        statsA = persist.tile([128, 3 * NCK], F32, tag="gn2sA")
        statsB = persist.tile([128, 3 * NCK], F32, tag="gn2sB")
        for c in range(NCK):
            cc = slice(c * CK, (c + 1) * CK)
            for m in range(3):
                msl = slice(m * 128, (m + 1) * 128)
                gsl = slice(384 + m * 128, 384 + (m + 1) * 128)
                g_ps = ps_mm.tile([128, CK], F32, tag="mm")
                nc.tensor.matmul(g_ps[:], sb['c2wf' + sfx][:, gsl], yf[:, cc],
                                 start=True, stop=False)
                nc.tensor.matmul(g_ps[:], sb['c2wb' + sfx][:, gsl], yb[:, cc],
                                 start=False, stop=True)
                sig = small.tile([128, CK], BF16, tag="sig")
                nc.scalar.activation(out=sig[:], in_=g_ps[:], func=AF.Sigmoid,
                                     bias=sb['c2bg' + sfx][:, m:m + 1])
                a_ps = ps_mm.tile([128, CK], F32, tag="mm")
                nc.tensor.matmul(a_ps[:], sb['c2wf' + sfx][:, msl], yf[:, cc],
                                 start=True, stop=False)
                nc.tensor.matmul(a_ps[:], sb['c2wb' + sfx][:, msl], yb[:, cc],
                                 start=False, stop=True)
                nc.vector.scalar_tensor_tensor(
                    out=glu[m][:, cc], in0=a_ps[:],
                    scalar=sb['c2ba' + sfx][:, m:m + 1], in1=sig[:],
                    op0=OP.add, op1=OP.mult,
                    accum_out=statsA[:, m * NCK + c:m * NCK + c + 1])
                sq = small.tile([128, CK], F32, tag="sqA", bufs=1)
                nc.scalar.activation(out=sq[:], in_=glu[m][:, cc],
                                     func=AF.Square,
                                     accum_out=statsB[:, m * NCK + c:m * NCK + c + 1])

        redB = tiny.tile([128, 2], F32, tag="gn2red")
        tot2_ps = ps_st.tile([1, 2], F32, tag="stat")
        for m in range(3):
            nc.vector.tensor_reduce(out=redB[:, 0:1], in_=statsA[:, m * NCK:(m + 1) * NCK],
                                    axis=mybir.AxisListType.X, op=OP.add)
            nc.vector.tensor_reduce(out=redB[:, 1:2], in_=statsB[:, m * NCK:(m + 1) * NCK],
                                    axis=mybir.AxisListType.X, op=OP.add)
            nc.tensor.matmul(tot2_ps[:], sb['ones128'][:], redB[:],
                             start=(m == 0), stop=(m == 2))
        _gn_finalize(nc, tiny, ps_st, sb, tot2_ps, CH * T, 128, "gn2", gnstate)
        svecs = []
        for m in range(3):
            sv = _gn_scale_bias(nc, tiny, sb['gn2w' + sfx][:, m:m + 1],
                                sb['gn2b' + sfx][:, m:m + 1], 128, f"gn2_{m}",
                                gnstate, base="gn2",
                                lsc=sb['lsc' + sfx][:, m:m + 1])
            svecs.append(sv)

        # ---------------- Phase G: v, residual, next-layer input ---------
        if li == 0:
            v0_buf = [persist.tile([128, T], BF16, tag=f"v0_{m}", name=f"v0_{m}") for m in range(3)]
            x1_dram = dram.tile([CH, T], BF16, tag="x1")
            for m in range(3):
                s2, t2 = svecs[m]
                for c in range(NCK):
                    cc = slice(c * CK, (c + 1) * CK)
                    nc.vector.tensor_scalar(
                        out=v0_buf[m][:, cc], in0=glu[m][:, cc],
                        scalar1=s2[:, 0:1], scalar2=t2[:, 0:1],
                        op0=OP.mult, op1=OP.add)
                    xr = stream.tile([128, CK], F32, tag="xres", bufs=2)
                    nc.sync.dma_start(out=xr[:],
                                      in_=io['x_res'][m * 128:(m + 1) * 128, cc])
                    x1c = small.tile([128, CK], BF16, tag="x1c", bufs=2)
                    nc.vector.tensor_add(x1c[:], xr[:], v0_buf[m][:, cc])
                    nc.sync.dma_start(out=x1_dram[m * 128:(m + 1) * 128, cc],
                                      in_=x1c[:])
        else:
            for m in range(3):
                s2, t2 = svecs[m]
                for c in range(NCK):
                    cc = slice(c * CK, (c + 1) * CK)
                    v1 = small.tile([128, CK], BF16, tag="v1", bufs=2)
                    nc.vector.tensor_scalar(
                        out=v1[:], in0=glu[m][:, cc],
                        scalar1=s2[:, 0:1], scalar2=t2[:, 0:1],
                        op0=OP.mult, op1=OP.add)
                    vs = small.tile([128, CK], F32, tag="vsum", bufs=2)
                    nc.vector.tensor_add(vs[:], v1[:], v0_buf[m][:, cc])
                    xr = stream.tile([128, CK], F32, tag="xres", bufs=2)
                    nc.sync.dma_start(out=xr[:],
                                      in_=io['x_res'][m * 128:(m + 1) * 128, cc])
                    of = small.tile([128, CK], F32, tag="ofin", bufs=2)
                    nc.vector.tensor_add(of[:], vs[:], xr[:])
                    nc.sync.dma_start(out=io['out'][m * 128:(m + 1) * 128, cc],
                                      in_=of[:])
    ctx.close()


def _gn_finalize(nc, tiny, ps_st, sb, tot_ps, nelem, parts, tag, gnstate):
    """psum [2,1] (sum, sumsq) -> broadcast sbuf [parts,2] = (mean, rstd)."""
    st = tiny.tile([1, 2], F32, tag=tag + "_st")
    nc.scalar.mul(out=st[:], in_=tot_ps[:], mul=1.0 / nelem)
    msq = tiny.tile([1, 1], F32, tag=tag + "_msq")
    nc.scalar.square(out=msq[:], in_=st[:, 0:1])
    var = tiny.tile([1, 1], F32, tag=tag + "_var")
    nc.vector.tensor_sub(var[:], st[:, 1:2], msq[:])
    eps = tiny.tile([1, 1], F32, tag=tag + "_eps")
    nc.vector.memset(eps[:], EPS)
    nc.scalar.activation(out=var[:], in_=var[:], func=AF.Sqrt, bias=eps[:, 0:1])
    nc.vector.reciprocal(out=var[:], in_=var[:])
    mr = tiny.tile([1, 2], F32, tag=tag + "_mr")
    nc.vector.tensor_copy(out=mr[:, 0:1], in_=st[:, 0:1])
    nc.vector.tensor_copy(out=mr[:, 1:2], in_=var[:])
    bc_ps = ps_st.tile([parts, 2], F32, tag="stat")
    nc.tensor.matmul(bc_ps[:], sb['onesrow'][:, 0:parts], mr[:],
                     start=True, stop=True)
    bc = tiny.tile([parts, 2], F32, tag=tag + "_bc")
    nc.vector.tensor_copy(out=bc[:], in_=bc_ps[:])
    gnstate[tag] = bc


def _gn_scale_bias(nc, tiny, w_ap, b_ap, parts, tag, gnstate, base=None, lsc=None):
    """out = in*s + t  ==  (in - mean)*rstd*w + b, optionally *lsc folded."""
    bc = gnstate[base or tag]
    s = tiny.tile([parts, 1], F32, tag=tag + "_s")
    nc.vector.tensor_mul(s[:], w_ap[:, 0:1], bc[:, 1:2])
    tneg = tiny.tile([parts, 1], F32, tag=tag + "_tn")
    nc.vector.scalar_tensor_tensor(
        out=tneg[:], in0=bc[:, 0:1], scalar=s[:, 0:1], in1=b_ap[:, 0:1],
        op0=OP.mult, op1=OP.subtract)
    if lsc is not None:
        nc.vector.tensor_mul(s[:], s[:], lsc[:, 0:1])
        nc.vector.tensor_mul(tneg[:], tneg[:], lsc[:, 0:1])
    t = tiny.tile([parts, 1], F32, tag=tag + "_t")
    nc.vector.tensor_scalar_mul(out=t[:], in0=tneg[:], scalar1=-1.0)
    return s, t


# ---------------------------------------------------------------------------
# entry point
# ---------------------------------------------------------------------------

_CACHED = {}


def kernel(x, params):
    x = np.asarray(x, np.float32)
    cores = _prep_inputs(x, params)
    if 'nc' not in _CACHED:
        _CACHED['nc'] = _build_program()
    nc = _CACHED['nc']
    res = run_bass_kernel_spmd(nc, cores, core_ids=list(range(N_CORES)))
    out = np.stack([res.results[0]['out'], res.results[4]['out']])
    return out.astype(np.float32)


if __name__ == '__main__':
    import jax
    jax.config.update('jax_platforms', 'cpu')
    import sys
    sys.path.insert(0, '/root/problem')
    import reference
    inputs = reference.setup_inputs()
    expected = np.asarray(reference.reference(**inputs))
    got = kernel(np.asarray(inputs['x']),
                 jax.tree.map(np.asarray, inputs['params']))
    err = np.abs(got - expected)
    print('max abs err', err.max(), 'out scale', np.abs(expected).max())
    print('rel fro', np.linalg.norm(got - expected) / np.linalg.norm(expected))


# revision 34
# speedup vs baseline: 115.5736x; 1.0665x over previous
"""Trainium2 Bass kernel for nn_CompRes (2-layer dilated-conv + BiMamba blocks).

Sharding: 8 cores = (batch b in 2) x (direction fwd/bwd) x (d-half of the mamba
inner channels). Every selective-scan recurrence is core-local (independent per
(b, dir, d, n)), running along time via the DVE tensor_tensor_scan instruction.
The backward direction uses host-reversed inputs + flipped conv taps so one
uniform SPMD program runs on all 8 cores. Two AllGathers per layer exchange
partial xproj / out-proj contractions. Pre/post-mamba stages are cheap matmuls
computed replicated per sample.

Numerics: residual path fp32; branch mostly bf16 (LayerScale 1e-4 damps branch
noise). Scan state is fp32 internally with bf16 operands.

Self-contained: hardcodes all shapes; reads no files.
"""
import numpy as np
import ml_dtypes

import concourse.bass as bass
import concourse.tile as tile
from concourse import mybir
from concourse.vector_clock import ScopedClock
from concourse.bass_utils import run_bass_kernel_spmd

F32 = mybir.dt.float32
BF16 = mybir.dt.bfloat16
AF = mybir.ActivationFunctionType
OP = mybir.AluOpType
BF = ml_dtypes.bfloat16

CH, HID, DI, DS, DR, DCONV = 384, 96, 192, 16, 6, 4
B = 2
T = 4096
CK = 512                 # matmul/psum chunk
MEGA = 2048              # scan mega-chunk
NCK = T // CK
NMC = T // MEGA
SUB = MEGA // CK
NG = 12                  # d-groups of 8 per core (96 local d's)
EPS = 1e-5
N_CORES = 8
DBC = DR + 2 * DS        # 38

_PATCHED = False


def _patch_tail_drain():
    """This walrus build rejects >2 sync waits on a CTRL instruction; split the
    TileContext tail-drain waits into one instruction each."""
    global _PATCHED
    if _PATCHED:
        return
    _PATCHED = True

    def _drain_and_barrier(self, tick_clock, wait_clock):
        nc = self.nc
        drain_inst = nc.sync.drain()
        wait_clock.add_sem_waits(
            drain_inst.ins, ScopedClock({None: tick_clock.global_clock})
        )
        si = drain_inst.ins.sync_info
        waits = list(si.on_wait)
        if len(waits) > 1:
            si.on_wait = []
            for w in waits:
                ni = nc.sync.nop(nofuse=True)
                ni.ins.sync_info = mybir.SyncInfo(on_wait=[w], on_update=[])
        nc.all_engine_barrier()
        assert self.sems is not None
        popped = nc._tile_sem_poison_stack.pop()
        assert popped is self._sem_poison
        nc.clear_and_free_semaphores(list(self.sems.allocated().values()))
        nc.all_engine_barrier()

    tile.TileContext._drain_and_barrier = _drain_and_barrier


# ---------------------------------------------------------------------------
# host-side input prep
# ---------------------------------------------------------------------------

def _prep_inputs(x, params):
    x = np.asarray(x, np.float32)

    def bf(a):
        return np.ascontiguousarray(np.asarray(a, np.float32).astype(BF))

    def f32(a):
        return np.ascontiguousarray(np.asarray(a, np.float32))

    r96 = np.zeros((128, NG, 96), np.float32)
    for n in range(16):
        for dl in range(8):
            for g in range(NG):
                r96[n * 8 + dl, g, 8 * g + dl] = 1.0
    ones96 = np.ones((96, 1), np.float32)
    ones128 = np.ones((128, 1), np.float32)
    onesrow = np.ones((1, 128), np.float32)

    cores = []
    for b in range(B):
        for dirflag in (0, 1):
            for dh in (0, 1):
                d0 = dh * 96
                dsl = slice(d0, d0 + 96)
                inp = {
                    'x_res': f32(x[b]),
                    'x_scan': bf(x[b][:, ::-1] if dirflag else x[b]),
                    'r96': bf(r96),
                    'ones96': ones96, 'ones128': ones128, 'onesrow': onesrow,
                }
                for li, lname in enumerate(['layer0', 'layer1']):
                    lp = params[lname]
                    mp = lp['mb'] if dirflag else lp['mf']
                    sfx = f'_l{li}'
                    w = np.asarray(lp['c1w'], np.float32)      # [96, 384, 3]
                    wf = w[:, :, ::-1]
                    if li == 0:
                        wA, wB = (wf if dirflag else w), None
                    else:
                        wA = np.zeros_like(w) if dirflag else w
                        wB = wf if dirflag else np.zeros_like(w)

                    def packc1(wm):
                        # [128, 9, 96]: slot (k*3+kt) holds lhsT rows kt*128..
                        out = np.zeros((128, 9, 96), np.float32)
                        for k in range(3):
                            lhsT = wm[:, :, k].T              # [384, 96]
                            for kt in range(3):
                                out[:, k * 3 + kt, :] = lhsT[kt * 128:(kt + 1) * 128]
                        return out

                    inp['c1wA' + sfx] = bf(packc1(wA))
                    if li == 1:
                        inp['c1wB' + sfx] = bf(packc1(wB))
                    inp['c1b' + sfx] = f32(np.asarray(lp['c1b'])[:, None])
                    inp['gn1w' + sfx] = f32(np.asarray(lp['gn1w'])[:, None])
                    inp['gn1b' + sfx] = f32(np.asarray(lp['gn1b'])[:, None])

                    in_w = np.asarray(mp['in_w'], np.float32)  # [384, 96]
                    inp['inwx' + sfx] = bf(in_w[dsl].T)
                    inp['inwz' + sfx] = bf(in_w[192 + d0:192 + d0 + 96].T)
                    inp['convw' + sfx] = f32(np.asarray(mp['conv_w'])[dsl, 0])
                    inp['convb' + sfx] = f32(np.asarray(mp['conv_b'])[dsl][:, None])
                    xpw = np.asarray(mp['xproj_w'], np.float32)[:, dsl].T.copy()
                    xpw[:, DR:DR + DS] *= -1.0   # dtx carries the other -1
                    inp['xpw' + sfx] = bf(xpw)
                    cw4 = np.asarray(mp['conv_w'], np.float32)[dsl, 0]  # [96,4]
                    cdiag = np.zeros((96, DCONV, 96), np.float32)
                    for j in range(DCONV):
                        np.fill_diagonal(cdiag[:, j, :], cw4[:, j])
                    inp['cdiag' + sfx] = bf(cdiag)
                    inp['dtw' + sfx] = bf(np.asarray(mp['dt_w'])[dsl].T)
                    inp['dtb' + sfx] = f32(-np.asarray(mp['dt_b'])[dsl][:, None])
                    # dt_buf holds ndt = -softplus(raw); fold the sign here
                    A = np.exp(np.asarray(mp['A_log'], np.float32)[dsl])
                    acol = np.zeros((128, NG), np.float32)
                    for g in range(NG):
                        for n in range(16):
                            for dl in range(8):
                                acol[n * 8 + dl, g] = A[8 * g + dl, n]
                    inp['acol' + sfx] = acol
                    inp['dd' + sfx] = f32(np.asarray(mp['D'])[dsl][:, None])
                    inp['outw' + sfx] = bf(np.asarray(mp['out_w'])[:, dsl].T)

                    c2w = np.asarray(lp['c2w'], np.float32)[:, :, 0]
                    inp['c2wf' + sfx] = bf(c2w[:, 0:96].T)
                    inp['c2wb' + sfx] = bf(c2w[:, 96:192].T)
                    c2b = np.asarray(lp['c2b'], np.float32)
                    inp['c2ba' + sfx] = f32(c2b[0:384].reshape(3, 128).T)
                    inp['c2bg' + sfx] = f32(c2b[384:768].reshape(3, 128).T)
                    inp['gn2w' + sfx] = f32(np.asarray(lp['gn2w'], np.float32).reshape(3, 128).T)
                    inp['gn2b' + sfx] = f32(np.asarray(lp['gn2b'], np.float32).reshape(3, 128).T)
                    inp['lsc' + sfx] = f32(np.asarray(lp['scale'], np.float32).reshape(3, 128).T)
                cores.append(inp)
    return cores


# ---------------------------------------------------------------------------
# device program
# ---------------------------------------------------------------------------

def _declare_ios(nc):
    d = {}

    def di(name, shape, dt):
        d[name] = nc.dram_tensor(name, shape, dt, kind="ExternalInput").ap()

    di('x_res', [CH, T], F32)
    di('x_scan', [CH, T], BF16)
    di('r96', [128, NG, 96], BF16)
    di('ones96', [96, 1], F32)
    di('ones128', [128, 1], F32)
    di('onesrow', [1, 128], F32)
    for li in (0, 1):
        sfx = f'_l{li}'
        di('c1wA' + sfx, [128, 9, 96], BF16)
        if li == 1:
            di('c1wB' + sfx, [128, 9, 96], BF16)
        di('c1b' + sfx, [96, 1], F32)
        di('gn1w' + sfx, [96, 1], F32)
        di('gn1b' + sfx, [96, 1], F32)
        di('inwx' + sfx, [96, 96], BF16)
        di('inwz' + sfx, [96, 96], BF16)
        di('convw' + sfx, [96, 4], F32)
        di('cdiag' + sfx, [96, DCONV, 96], BF16)
        di('convb' + sfx, [96, 1], F32)
        di('xpw' + sfx, [96, DBC], BF16)
        di('dtw' + sfx, [6, 96], BF16)
        di('dtb' + sfx, [96, 1], F32)
        di('acol' + sfx, [128, NG], F32)
        di('dd' + sfx, [96, 1], F32)
        di('outw' + sfx, [96, 96], BF16)
        di('c2wf' + sfx, [96, 768], BF16)
        di('c2wb' + sfx, [96, 768], BF16)
        di('c2ba' + sfx, [128, 3], F32)
        di('c2bg' + sfx, [128, 3], F32)
        di('gn2w' + sfx, [128, 3], F32)
        di('gn2b' + sfx, [128, 3], F32)
        di('lsc' + sfx, [128, 3], F32)
    d['out'] = nc.dram_tensor('out', [CH, T], F32, kind="ExternalOutput").ap()
    return d


def _split_excess_waits(nc, max_waits=1):
    """This walrus build caps sync waits per instruction; hoist extras onto
    same-engine NoOps inserted just before the instruction."""
    uid = [0]
    for f in nc.m.functions:
        for bb in f.blocks:
            out = []
            for ins in bb.instructions:
                si = ins.sync_info
                if si is not None and len(si.on_wait) > max_waits:
                    waits = list(si.on_wait)
                    keep = waits[:max_waits]
                    extra = waits[max_waits:]
                    for i in range(0, len(extra), max_waits):
                        uid[0] += 1
                        nop = mybir.InstNoOp(
                            name=f"wsplit-{uid[0]}", engine=ins.engine,
                            ins=[], outs=[],
                            sync_info=mybir.SyncInfo(
                                on_wait=extra[i:i + max_waits], on_update=[]))
                        out.append(nop)
                    si.on_wait = keep
                out.append(ins)
            bb.instructions[:] = out


def _build_program(single=False):
    _patch_tail_drain()
    nc = bass.Bass(num_devices=1 if single else N_CORES)
    io = _declare_ios(nc)
    with tile.TileContext(nc) as tc:
        _emit(nc, tc, io, single=single)
    _split_excess_waits(nc)
    return nc


def _emit(nc, tc, io, single=False):
    from contextlib import ExitStack
    ctx = ExitStack()
    persist = ctx.enter_context(tc.tile_pool(name="persist", bufs=1))
    wpool = ctx.enter_context(tc.tile_pool(name="weights", bufs=1))
    stream = ctx.enter_context(tc.tile_pool(name="stream", bufs=3))
    mega = ctx.enter_context(tc.tile_pool(name="mega", bufs=2))
    mega1 = ctx.enter_context(tc.tile_pool(name="mega1", bufs=1))
    small = ctx.enter_context(tc.tile_pool(name="small", bufs=2))
    tiny = ctx.enter_context(tc.tile_pool(name="tiny", bufs=1))
    ps_mm = ctx.enter_context(tc.tile_pool(name="ps_mm", bufs=3, space="PSUM"))
    ps_st = ctx.enter_context(tc.tile_pool(name="ps_st", bufs=1, space="PSUM"))
    ps_y = ctx.enter_context(tc.tile_pool(name="ps_y", bufs=4, space="PSUM"))
    dram = ctx.enter_context(tc.tile_pool(name="dram", bufs=1, space="DRAM"))

    # ---- load weights/constants to SBUF once ----
    sb = {}
    for name, ap in io.items():
        if name in ('x_res', 'x_scan', 'out'):
            continue
        t_ = wpool.tile(list(ap.shape), ap.dtype, tag=name)
        nc.sync.dma_start(out=t_[:], in_=ap[:])
        sb[name] = t_

    pair_groups = [[0, 1], [2, 3], [4, 5], [6, 7]]
    b_groups = [[0, 1, 2, 3], [4, 5, 6, 7]]
    gnstate = {}

    v0_buf = None
    x1_dram = None

    for li in (0, 1):
        sfx = f'_l{li}'
        dil = 2 ** li

        # ---------------- Phase A: conv1 + GN1 stats ----------------
        u_buf = persist.tile([96, T], BF16, tag="u")
        stats = persist.tile([96, 2 * NCK], F32, tag="gn1stats")
        halo = dil
        for c in range(NCK):
            ups = ps_mm.tile([96, CK], F32, tag="mm")
            n_mm = 9 if li == 0 else 18
            mi = 0
            if li == 0:
                for kt in range(3):
                    xs = stream.tile([128, CK + 2 * halo], BF16, tag="xs", bufs=3)
                    lo, hi = c * CK - halo, c * CK + CK + halo
                    plo, phi = max(lo, 0), min(hi, T)
                    if lo < 0:
                        nc.vector.memset(xs[:, 0:(plo - lo)], 0.0)
                    if hi > T:
                        nc.vector.memset(xs[:, (phi - lo):(hi - lo)], 0.0)
                    nc.sync.dma_start(
                        out=xs[:, (plo - lo):(phi - lo)],
                        in_=io['x_scan'][kt * 128:(kt + 1) * 128, plo:phi])
                    for k in range(3):
                        nc.tensor.matmul(
                            ups[:], sb['c1wA' + sfx][:, k * 3 + kt, :],
                            xs[:, k * dil:k * dil + CK],
                            start=(mi == 0), stop=(mi == n_mm - 1))
                        mi += 1
            else:
                for kt in range(3):
                    lo, hi = c * CK - halo, c * CK + CK + halo
                    plo, phi = max(lo, 0), min(hi, T)
                    xs = stream.tile([128, CK + 2 * halo], BF16, tag="xs", bufs=3)
                    if lo < 0:
                        nc.vector.memset(xs[:, 0:(plo - lo)], 0.0)
                    if hi > T:
                        nc.vector.memset(xs[:, (phi - lo):(hi - lo)], 0.0)
                    nc.sync.dma_start(
                        out=xs[:, (plo - lo):(phi - lo)],
                        in_=x1_dram[kt * 128:(kt + 1) * 128, plo:phi])
                    # reversed frame handled via reversed SBUF AP on the
                    # matmul rhs; tile holds x1 cols [rlo, rhi) contiguously.
                    rlo, rhi = T - hi, T - lo
                    rplo, rphi = max(rlo, 0), min(rhi, T)
                    xr_ = stream.tile([128, CK + 2 * halo], BF16, tag="xsr", bufs=3)
                    if rlo < 0:
                        nc.vector.memset(xr_[:, 0:(rplo - rlo)], 0.0)
                    if rhi > T:
                        nc.vector.memset(xr_[:, (rphi - rlo):(rhi - rlo)], 0.0)
                    nc.sync.dma_start(
                        out=xr_[:, (rplo - rlo):(rphi - rlo)],
                        in_=x1_dram[kt * 128:(kt + 1) * 128, rplo:rphi])
                    for k in range(3):
                        nc.tensor.matmul(
                            ups[:], sb['c1wA' + sfx][:, k * 3 + kt, :],
                            xs[:, k * dil:k * dil + CK],
                            start=(mi == 0), stop=False)
                        mi += 1
                        roff = halo - (k - 1) * dil
                        nc.tensor.matmul(
                            ups[:], sb['c1wB' + sfx][:, k * 3 + kt, :],
                            xr_[:, roff:roff + CK][:, ::-1],
                            start=False, stop=(mi == n_mm - 1))
                        mi += 1
            nc.scalar.activation(
                out=u_buf[:, c * CK:(c + 1) * CK], in_=ups[:],
                func=AF.Identity, bias=sb['c1b' + sfx][:, 0:1],
                accum_out=stats[:, c:c + 1])
            sq = small.tile([96, CK], F32, tag="sqA", bufs=1)
            nc.scalar.activation(
                out=sq[:], in_=u_buf[:, c * CK:(c + 1) * CK],
                func=AF.Square, accum_out=stats[:, NCK + c:NCK + c + 1])

        red = tiny.tile([96, 2], F32, tag="gn1red")
        nc.vector.tensor_reduce(out=red[:, 0:1], in_=stats[:, 0:NCK],
                                axis=mybir.AxisListType.X, op=OP.add)
        nc.vector.tensor_reduce(out=red[:, 1:2], in_=stats[:, NCK:2 * NCK],
                                axis=mybir.AxisListType.X, op=OP.add)
        tot_ps = ps_st.tile([1, 2], F32, tag="stat")
        nc.tensor.matmul(tot_ps[:], sb['ones96'][:], red[:], start=True, stop=True)
        _gn_finalize(nc, tiny, ps_st, sb, tot_ps, 96 * T, 96, "gn1", gnstate)
        s_gn, t_gn = _gn_scale_bias(nc, tiny, sb['gn1w' + sfx], sb['gn1b' + sfx],
                                    96, "gn1", gnstate)
        for c in range(NCK):
            nc.scalar.activation(
                out=u_buf[:, c * CK:(c + 1) * CK],
                in_=u_buf[:, c * CK:(c + 1) * CK],
                func=AF.Gelu, bias=t_gn[:, 0:1], scale=s_gn[:, 0:1])

        # ---------------- Phase B: in-proj, dconv, xproj ----------------
        zs_buf = persist.tile([96, T], BF16, tag="zs")
        xm_buf = persist.tile([96, T + DCONV - 1], BF16, tag="xm")
        xms_buf = persist.tile([96, T], BF16, tag="xms")
        dbc_part = persist.tile([DBC, T], BF16, tag="dt", name="dbc_part")
        nc.vector.memset(xm_buf[:, 0:DCONV - 1], 0.0)
        for c in range(NCK):
            cc = slice(c * CK, (c + 1) * CK)
            xm_ps = ps_mm.tile([96, CK], F32, tag="mm")
            nc.tensor.matmul(xm_ps[:], sb['inwx' + sfx][:], u_buf[:, cc],
                             start=True, stop=True)
            nc.vector.tensor_copy(
                out=xm_buf[:, DCONV - 1 + c * CK:DCONV - 1 + (c + 1) * CK],
                in_=xm_ps[:])
            z_ps = ps_mm.tile([96, CK], F32, tag="mm")
            nc.tensor.matmul(z_ps[:], sb['inwz' + sfx][:], u_buf[:, cc],
                             start=True, stop=True)
            nc.scalar.activation(out=zs_buf[:, cc], in_=z_ps[:], func=AF.Silu)
            # depthwise conv as 4 diag-matmuls accumulating in psum
            dc_ps = ps_mm.tile([96, CK], F32, tag="mm")
            for j in range(DCONV):
                nc.tensor.matmul(dc_ps[:], sb['cdiag' + sfx][:, j, :],
                                 xm_buf[:, c * CK + j:c * CK + j + CK],
                                 start=(j == 0), stop=(j == DCONV - 1))
            nc.scalar.activation(out=xms_buf[:, cc], in_=dc_ps[:],
                                 func=AF.Silu, bias=sb['convb' + sfx][:, 0:1])
            xp_ps = ps_mm.tile([DBC, CK], F32, tag="mm")
            nc.tensor.matmul(xp_ps[:], sb['xpw' + sfx][:], xms_buf[:, cc],
                             start=True, stop=True)
            nc.vector.tensor_copy(out=dbc_part[:, cc], in_=xp_ps[:])

        # ---------------- AllGather #1: dbc pair partials ----------------
        dbc_in = dram.tile([DBC, T], BF16, tag="dbc_in")
        dbc_out = dram.tile([2, DBC, T], BF16, tag="dbc_out")
        nc.gpsimd.dma_start(out=dbc_in[:], in_=dbc_part[:])
        if single:
            for _sl in range(2):
                nc.gpsimd.dma_start(out=dbc_out[_sl, :, :], in_=dbc_in[:])
        else:
            nc.gpsimd.collective_compute(
                "AllGather", OP.bypass, replica_groups=pair_groups,
                ins=[dbc_in[:]], outs=[dbc_out[:]])
        ga = persist.tile([96, T], BF16, tag="u", name="ga1")
        gb = persist.tile([96, T], BF16, tag="xm", name="gb1")
        nc.sync.dma_start(out=ga[0:DBC, :], in_=dbc_out[0, :, :])
        nc.sync.dma_start(out=gb[0:DBC, :], in_=dbc_out[1, :, :])
        dbc = persist.tile([DBC, T], BF16, tag="dbc")
        nc.vector.tensor_add(dbc[:], ga[0:DBC, :], gb[0:DBC, :])
        dbc_dram = dram.tile([DBC, T], BF16, tag="dbc_dram")
        nc.sync.dma_start(out=dbc_dram[:], in_=dbc[:])

        # ---------------- Phase C: dt, dtx ----------------
        dt_buf = persist.tile([96, T], BF16, tag="dt", name="dt_buf")
        dtx_buf = persist.tile([96, T], BF16, tag="dtx")
        # no Softplus act table on this HW: ndt = ln(sigmoid(-raw)) = -softplus
        # (sign folded into acol / q16b host constants). fp32 intermediate.
        nlo = (NCK + 1) // 2
        sig_lo = persist.tile([96, nlo * CK], F32, tag="u", name="sig_lo")
        sig_hi = (persist.tile([96, (NCK - nlo) * CK], F32, tag="xm",
                               name="sig_hi") if NCK > nlo else None)

        def _sig_slice(c):
            if c < nlo:
                return sig_lo[:, c * CK:(c + 1) * CK]
            return sig_hi[:, (c - nlo) * CK:(c - nlo + 1) * CK]

        for c in range(NCK):
            dt_ps = ps_mm.tile([96, CK], F32, tag="mm")
            nc.tensor.matmul(dt_ps[:], sb['dtw' + sfx][:], dbc[0:DR, cc := slice(c * CK, (c + 1) * CK)],
                             start=True, stop=True)
            nc.scalar.activation(out=_sig_slice(c), in_=dt_ps[:],
                                 func=AF.Sigmoid, scale=-1.0,
                                 bias=sb['dtb' + sfx][:, 0:1])
        dt_dram = dram.tile([96, T], BF16, tag="dt_dram")
        dtx_dram = dram.tile([96, T], BF16, tag="dtx_dram")
        for c in range(NCK):
            cc = slice(c * CK, (c + 1) * CK)
            nc.scalar.activation(out=dt_buf[:, cc], in_=_sig_slice(c), func=AF.Ln)
            nc.vector.tensor_mul(dtx_buf[:, cc], dt_buf[:, cc], xms_buf[:, cc])
            nc.gpsimd.dma_start(out=dt_dram[:, cc], in_=dt_buf[:, cc])
            nc.gpsimd.dma_start(out=dtx_dram[:, cc], in_=dtx_buf[:, cc])

        # ---------------- Phase D/E: scan tiles, gate, out-proj ----------
        yo_buf = persist.tile([96, T], BF16, tag="dtx", name="yo_buf")
        carry = persist.tile([128, NG], F32, tag="carry")
        dbc_b = dbc_dram[:]
        dt_b = dt_dram[:]
        dtx_b = dtx_dram[:]
        for mc in range(NMC):
            off = mc * MEGA
            brep = mega1.tile([128, MEGA], BF16, tag="brep")
            crep = mega1.tile([128, MEGA], BF16, tag="crep")
            nc.sync.dma_start(out=brep[:], in_=bass.AP(
                tensor=dbc_b.tensor, offset=dbc_b.offset + DR * T + off,
                ap=[[T, 16], [0, 8], [1, MEGA]]))
            nc.sync.dma_start(out=crep[:], in_=bass.AP(
                tensor=dbc_b.tensor, offset=dbc_b.offset + (DR + DS) * T + off,
                ap=[[T, 16], [0, 8], [1, MEGA]]))
            yps_list = {}
            for s in range(SUB):
                yps_list[mc * SUB + s] = ps_y.tile([96, CK], F32, tag="y", name=f"yps_{mc}_{s}")
            for g in range(NG):
                dA = mega.tile([128, MEGA], BF16, tag="dA", bufs=3)
                dBx = mega.tile([128, MEGA], BF16, tag="dBx", bufs=3)
                nc.scalar.dma_start(out=dA[:], in_=bass.AP(
                    tensor=dt_b.tensor, offset=dt_b.offset + (8 * g) * T + off,
                    ap=[[0, 16], [T, 8], [1, MEGA]]))
                nc.scalar.activation(out=dA[:], in_=dA[:], func=AF.Exp,
                                     scale=sb['acol' + sfx][:, g:g + 1])
                nc.scalar.dma_start(out=dBx[:], in_=bass.AP(
                    tensor=dtx_b.tensor, offset=dtx_b.offset + (8 * g) * T + off,
                    ap=[[0, 16], [T, 8], [1, MEGA]]))
                nc.vector.tensor_mul(dBx[:], dBx[:], brep[:])
                h = mega.tile([128, MEGA], BF16, tag="h")
                init = 0.0 if mc == 0 else carry[:, g:g + 1]
                nc.vector.tensor_tensor_scan(
                    out=h[:], data0=dA[:], data1=dBx[:], initial=init,
                    op0=OP.mult, op1=OP.add)
                nc.vector.tensor_copy(out=carry[:, g:g + 1],
                                      in_=h[:, MEGA - 1:MEGA])
                hc = mega.tile([128, MEGA], BF16, tag="hc", name=f"hc_{mc}_{g}")
                if g % 2 == 1:
                    nc.gpsimd.tensor_mul(hc[:], h[:], crep[:])
                else:
                    nc.vector.tensor_mul(hc[:], h[:], crep[:])
                for s in range(SUB):
                    c = mc * SUB + s
                    sc = slice(s * CK, (s + 1) * CK)
                    nc.tensor.matmul(yps_list[c][:],
                                     sb['r96'][:, g, :], hc[:, sc],
                                     start=(g == 0), stop=(g == NG - 1))
            for s in range(SUB):
                c = mc * SUB + s
                cc = slice(c * CK, (c + 1) * CK)
                yt = small.tile([96, CK], BF16, tag="ygate")
                nc.vector.scalar_tensor_tensor(
                    out=yt[:], in0=xms_buf[:, cc], scalar=sb['dd' + sfx][:, 0:1],
                    in1=yps_list[c][:], op0=OP.mult, op1=OP.add)
                nc.vector.tensor_mul(yt[:], yt[:], zs_buf[:, cc])
                yo_ps = ps_mm.tile([96, CK], F32, tag="mm")
                nc.tensor.matmul(yo_ps[:], sb['outw' + sfx][:], yt[:],
                                 start=True, stop=True)
                nc.scalar.copy(out=yo_buf[:, cc], in_=yo_ps[:])

        # ---------------- AllGather #2: yo batch partials ----------------
        yo_in = dram.tile([96, T], BF16, tag="yo_in")
        yo_out = dram.tile([4, 96, T], BF16, tag="yo_out")
        nc.gpsimd.dma_start(out=yo_in[:], in_=yo_buf[:])
        if single:
            for _sl in range(4):
                nc.gpsimd.dma_start(out=yo_out[_sl, :, :], in_=yo_in[:])
        else:
            nc.gpsimd.collective_compute(
                "AllGather", OP.bypass, replica_groups=b_groups,
                ins=[yo_in[:]], outs=[yo_out[:]])
        yf = persist.tile([96, T], BF16, tag="u", name="yf")
        yb = persist.tile([96, T], BF16, tag="zs", name="yb")
        ga3 = persist.tile([96, T], BF16, tag="xm", name="ga3")
        gb3 = persist.tile([96, T], BF16, tag="dtx", name="gb3")
        nc.sync.dma_start(out=ga3[:], in_=yo_out[0, :, :])
        nc.sync.dma_start(out=gb3[:], in_=yo_out[1, :, :])
        nc.vector.tensor_add(yf[:], ga3[:], gb3[:])
        ga4 = persist.tile([96, T], BF16, tag="xm", name="ga4")
        gb4 = persist.tile([96, T], BF16, tag="dtx", name="gb4")
        nc.sync.dma_start(out=ga4[:], in_=yo_out[2, :, :])
        nc.sync.dma_start(out=gb4[:], in_=yo_out[3, :, :])
        nc.vector.tensor_add(yb[:], ga4[:], gb4[:])

        # ---------------- Phase F: conv2 + GLU + GN2 ----------------
        glu = [persist.tile([128, T], BF16, tag=f"glu{m}", name=f"glu{m}") for m in range(3)]
        statsA = persist.tile([128, 3 * NCK], F32, tag="gn2sA")
        statsB = persist.tile([128, 3 * NCK], F32, tag="gn2sB")
        for c in range(NCK):
            cc = slice(c * CK, (c + 1) * CK)
            for m in range(3):
                msl = slice(m * 128, (m + 1) * 128)
                gsl = slice(384 + m * 128, 384 + (m + 1) * 128)
                g_ps = ps_mm.tile([128, CK], F32, tag="mm")
                nc.tensor.matmul(g_ps[:], sb['c2wf' + sfx][:, gsl], yf[:, cc],
                                 start=True, stop=False)
                nc.tensor.matmul(g_ps[:], sb['c2wb' + sfx][:, gsl], yb[:, cc],
                                 start=False, stop=True)
                sig = small.tile([128, CK], BF16, tag="sig")
                nc.scalar.activation(out=sig[:], in_=g_ps[:], func=AF.Sigmoid,
                                     bias=sb['c2bg' + sfx][:, m:m + 1])
                a_ps = ps_mm.tile([128, CK], F32, tag="mm")
                nc.tensor.matmul(a_ps[:], sb['c2wf' + sfx][:, msl], yf[:, cc],
                                 start=True, stop=False)
                nc.tensor.matmul(a_ps[:], sb['c2wb' + sfx][:, msl], yb[:, cc],
                                 start=False, stop=True)
                nc.vector.scalar_tensor_tensor(
                    out=glu[m][:, cc], in0=a_ps[:],
                    scalar=sb['c2ba' + sfx][:, m:m + 1], in1=sig[:],
                    op0=OP.add, op1=OP.mult,
                    accum_out=statsA[:, m * NCK + c:m * NCK + c + 1])
                sq = small.tile([128, CK], BF16, tag="sqF", bufs=1)
                nc.vector.scalar_tensor_tensor(
                    out=sq[:], in0=a_ps[:],
                    scalar=sb['c2ba' + sfx][:, m:m + 1], in1=glu[m][:, cc],
                    op0=OP.add, op1=OP.mult,
                    accum_out=statsB[:, m * NCK + c:m * NCK + c + 1])

        redB = tiny.tile([128, 2], F32, tag="gn2red")
        tot2_ps = ps_st.tile([1, 2], F32, tag="stat")
        for m in range(3):
            nc.vector.tensor_reduce(out=redB[:, 0:1], in_=statsA[:, m * NCK:(m + 1) * NCK],
                                    axis=mybir.AxisListType.X, op=OP.add)
            nc.vector.tensor_reduce(out=redB[:, 1:2], in_=statsB[:, m * NCK:(m + 1) * NCK],
                                    axis=mybir.AxisListType.X, op=OP.add)
            nc.tensor.matmul(tot2_ps[:], sb['ones128'][:], redB[:],
                             start=(m == 0), stop=(m == 2))
        _gn_finalize(nc, tiny, ps_st, sb, tot2_ps, CH * T, 128, "gn2", gnstate)
        svecs = []
        for m in range(3):
            sv = _gn_scale_bias(nc, tiny, sb['gn2w' + sfx][:, m:m + 1],
                                sb['gn2b' + sfx][:, m:m + 1], 128, f"gn2_{m}",
                                gnstate, base="gn2",
                                lsc=sb['lsc' + sfx][:, m:m + 1])
            svecs.append(sv)

        # ---------------- Phase G: v, residual, next-layer input ---------
        if li == 0:
            v0_buf = [persist.tile([128, T], BF16, tag=f"v0_{m}", name=f"v0_{m}") for m in range(3)]
            x1_dram = dram.tile([CH, T], BF16, tag="x1")
            CG = 1024
            for m in range(3):
                s2, t2 = svecs[m]
                for c in range(T // CG):
                    cc = slice(c * CG, (c + 1) * CG)
                    nc.vector.tensor_scalar(
                        out=v0_buf[m][:, cc], in0=glu[m][:, cc],
                        scalar1=s2[:, 0:1], scalar2=t2[:, 0:1],
                        op0=OP.mult, op1=OP.add)
                    xr = stream.tile([128, CG], F32, tag="xres", bufs=2)
                    nc.sync.dma_start(out=xr[:],
                                      in_=io['x_res'][m * 128:(m + 1) * 128, cc])
                    x1c = small.tile([128, CG], BF16, tag="x1c", bufs=2)
                    nc.vector.tensor_add(x1c[:], xr[:], v0_buf[m][:, cc])
                    nc.sync.dma_start(out=x1_dram[m * 128:(m + 1) * 128, cc],
                                      in_=x1c[:])
        else:
            CG = 1024
            for m in range(3):
                s2, t2 = svecs[m]
                for c in range(T // CG):
                    cc = slice(c * CG, (c + 1) * CG)
                    v1 = small.tile([128, CG], BF16, tag="v1", bufs=1)
                    nc.vector.tensor_scalar(
                        out=v1[:], in0=glu[m][:, cc],
                        scalar1=s2[:, 0:1], scalar2=t2[:, 0:1],
                        op0=OP.mult, op1=OP.add)
                    xr = stream.tile([128, CG], F32, tag="xres", bufs=2)
                    nc.sync.dma_start(out=xr[:],
                                      in_=io['x_res'][m * 128:(m + 1) * 128, cc])
                    of = small.tile([128, CG], F32, tag="ofin", bufs=2)
                    nc.vector.tensor_add(of[:], v1[:], v0_buf[m][:, cc])
                    nc.vector.tensor_add(of[:], of[:], xr[:])
                    nc.sync.dma_start(out=io['out'][m * 128:(m + 1) * 128, cc],
                                      in_=of[:])
    ctx.close()


def _gn_finalize(nc, tiny, ps_st, sb, tot_ps, nelem, parts, tag, gnstate):
    """psum [2,1] (sum, sumsq) -> broadcast sbuf [parts,2] = (mean, rstd)."""
    st = tiny.tile([1, 2], F32, tag=tag + "_st")
    nc.scalar.mul(out=st[:], in_=tot_ps[:], mul=1.0 / nelem)
    msq = tiny.tile([1, 1], F32, tag=tag + "_msq")
    nc.scalar.square(out=msq[:], in_=st[:, 0:1])
    var = tiny.tile([1, 1], F32, tag=tag + "_var")
    nc.vector.tensor_sub(var[:], st[:, 1:2], msq[:])
    eps = tiny.tile([1, 1], F32, tag=tag + "_eps")
    nc.vector.memset(eps[:], EPS)
    nc.scalar.activation(out=var[:], in_=var[:], func=AF.Sqrt, bias=eps[:, 0:1])
    nc.vector.reciprocal(out=var[:], in_=var[:])
    mr = tiny.tile([1, 2], F32, tag=tag + "_mr")
    nc.vector.tensor_copy(out=mr[:, 0:1], in_=st[:, 0:1])
    nc.vector.tensor_copy(out=mr[:, 1:2], in_=var[:])
    bc_ps = ps_st.tile([parts, 2], F32, tag="stat")
    nc.tensor.matmul(bc_ps[:], sb['onesrow'][:, 0:parts], mr[:],
                     start=True, stop=True)
    bc = tiny.tile([parts, 2], F32, tag=tag + "_bc")
    nc.vector.tensor_copy(out=bc[:], in_=bc_ps[:])
    gnstate[tag] = bc


def _gn_scale_bias(nc, tiny, w_ap, b_ap, parts, tag, gnstate, base=None, lsc=None):
    """out = in*s + t  ==  (in - mean)*rstd*w + b, optionally *lsc folded."""
    bc = gnstate[base or tag]
    s = tiny.tile([parts, 1], F32, tag=tag + "_s")
    nc.vector.tensor_mul(s[:], w_ap[:, 0:1], bc[:, 1:2])
    tneg = tiny.tile([parts, 1], F32, tag=tag + "_tn")
    nc.vector.scalar_tensor_tensor(
        out=tneg[:], in0=bc[:, 0:1], scalar=s[:, 0:1], in1=b_ap[:, 0:1],
        op0=OP.mult, op1=OP.subtract)
    if lsc is not None:
        nc.vector.tensor_mul(s[:], s[:], lsc[:, 0:1])
        nc.vector.tensor_mul(tneg[:], tneg[:], lsc[:, 0:1])
    t = tiny.tile([parts, 1], F32, tag=tag + "_t")
    nc.vector.tensor_scalar_mul(out=t[:], in0=tneg[:], scalar1=-1.0)
    return s, t


# ---------------------------------------------------------------------------
# entry point
# ---------------------------------------------------------------------------

_CACHED = {}


def kernel(x, params):
    x = np.asarray(x, np.float32)
    cores = _prep_inputs(x, params)
    if 'nc' not in _CACHED:
        _CACHED['nc'] = _build_program()
    nc = _CACHED['nc']
    res = run_bass_kernel_spmd(nc, cores, core_ids=list(range(N_CORES)))
    out = np.stack([res.results[0]['out'], res.results[4]['out']])
    return out.astype(np.float32)


if __name__ == '__main__':
    import jax
    jax.config.update('jax_platforms', 'cpu')
    import sys
    sys.path.insert(0, '/root/problem')
    import reference
    inputs = reference.setup_inputs()
    expected = np.asarray(reference.reference(**inputs))
    got = kernel(np.asarray(inputs['x']),
                 jax.tree.map(np.asarray, inputs['params']))
    err = np.abs(got - expected)
    print('max abs err', err.max(), 'out scale', np.abs(expected).max())
    print('rel fro', np.linalg.norm(got - expected) / np.linalg.norm(expected))
